# revision 1
# baseline (speedup 1.0000x reference)
"""GatedLTMMemory kernel for 8 Trainium2 NeuronCores.

Data-parallel over the 4096 flattened (B,N) tokens: 512 tokens per core.
Memory-slot tables and weights are replicated. The reference's per-selected-slot
projections (137 GFLOP) are replaced by projecting the slot tables once and
running a masked full-softmax over all S slots (exactly equivalent math).

Precision plan (fp32 matmuls run at 1/4 PE rate; float32r/bf16 at full rate):
  exact fp32 : selection path (q projection, slot norms, scores) — the top-32
               boundary gaps are ~1e-6 so this path cannot be rounded.
  float32r   : Kp/Vp/qh projections, attention logits, Wo/Wout epilogue
               (~1.6e-4 measured on HW).
  bf16       : softmax weights w = exp(att)*mask and the value table Vp
               (~2e-3; the denominators come from the same w so it cancels).

Emission order is chosen so the DVE top-k overlaps the PE Kp/Vp/qh
projections. SBUF pool tags are allocated statically, so dead tensors donate
their slots to later tensors (chains are noted inline). Host passes
weights/tables pre-transposed (layout prep only; no FLOPs moved to host).
"""

import numpy as np

import concourse.bacc as bacc
import concourse.mybir as mybir
import concourse.tile as tile
from concourse.bass import ds, ts
from concourse.bass_utils import run_bass_kernel_spmd
from concourse.masks import make_identity

B, N, QD, D, S, H, K = 4, 1024, 320, 512, 1024, 8, 32
DH = D // H
EPS = 1e-5
P = 128
T = 512                       # tokens per core
NCORES = 8
NT = T // P                   # 4 token tiles
ND = D // P                   # 4 contraction chunks over D
NS = S // P                   # 8 slot tiles
NEG = -1e30
QD_TILES = [(0, 128), (128, 128), (256, 64)]

f32 = mybir.dt.float32
f32r = mybir.dt.float32r
bf16 = mybir.dt.bfloat16
AF = mybir.ActivationFunctionType
OP = mybir.AluOpType

_CACHE: dict = {}


def _build_nc():
    nc = bacc.Bacc("TRN2", target_bir_lowering=False, debug=False)

    dr = {}

    def din(name, shape, dt_):
        dr[name] = nc.dram_tensor(name, shape, dt_, kind="ExternalInput")

    din("queryT", (QD, T), f32)
    din("WqpT", (QD, D), f32)
    din("WqT", (D, D), f32r)
    din("WkT", (D, D), f32r)
    din("WvT", (D, D), f32r)
    din("WoT", (D, D), f32r)
    din("WoutT", (D, QD), f32r)
    din("memkT", (D, S), f32)
    din("memvT", (D, S), f32)
    din("ln_g", (D,), f32)
    din("ln_b", (D,), f32)
    din("bout", (384,), f32)
    out_dram = nc.dram_tensor("outT", (QD, T), f32, kind="ExternalOutput")

    with tile.TileContext(nc) as tc:
        with (
            tc.tile_pool(name="const", bufs=1) as const,
            tc.tile_pool(name="main", bufs=1) as main,
            tc.tile_pool(name="scr2", bufs=2) as scr2,
            tc.tile_pool(name="scr4", bufs=8) as scr4,
            tc.tile_pool(name="psA", bufs=2, space="PSUM") as psA,
            tc.tile_pool(name="psB", bufs=1, space="PSUM") as psB,
            tc.tile_pool(name="psmm", bufs=4, space="PSUM") as psmm,
            nc.allow_low_precision(reason="validated f32r/bf16 paths"),
        ):
            # ---------- constants ----------
            ident = const.tile([P, P], bf16, tag="ident")
            make_identity(nc, ident)
            ident_f = const.tile([P, P], f32, tag="ident_f")
            make_identity(nc, ident_f)
            ones_col = const.tile([P, 1], f32, tag="ones_col")
            nc.vector.memset(ones_col, 1.0)
            ones_row = const.tile([1, P], f32, tag="ones_row")
            nc.vector.memset(ones_row, 1.0)
            # f32r half-ones rows for per-head-pair broadcast matmuls
            halfsel = const.tile([1, 2 * P], f32, tag="halfsel")
            nc.vector.memset(halfsel, 0.0)
            nc.vector.memset(halfsel[0:1, 64:192], 1.0)
            halfsel_r = const.tile([1, 2 * P], f32r, tag="halfsel_r")
            nc.scalar.copy(halfsel_r[:], halfsel[:])
            # halfsel layout: [0:64]=0, [64:192]=1, [192:256]=0
            ones_row_r = halfsel_r[0:1, 64:192]  # [1,128] all ones
            selA = halfsel_r[0:1, 128:256]       # [1,128]: ones x64, zeros x64
            selB = halfsel_r[0:1, 0:128]         # [1,128]: zeros x64, ones x64
            eps_tab = const.tile([P, 1], f32, tag="eps_tab")
            nc.vector.memset(eps_tab, 1e-12)
            eps_ln = const.tile([1, 1], f32, tag="eps_ln")
            nc.vector.memset(eps_ln, EPS)

            # ---------- weight loads ----------
            def load_rows(name, cols, row_tiles, tags, dt_):
                tiles = []
                for (off, sz), tag in zip(row_tiles, tags):
                    t_ = main.tile([sz, cols], dt_, tag=tag, name=f"ld_{tag}")
                    nc.sync.dma_start(t_[:], dr[name].ap()[ds(off, sz), :])
                    tiles.append(t_)
                return tiles

            d_rows = [(i * P, P) for i in range(ND)]
            qryT = load_rows("queryT", T, QD_TILES, ["qry0", "qry1", "qry2"], f32)
            wqpT = load_rows("WqpT", D, QD_TILES, ["wqp0", "wqp1", "wqp2"], f32)

            g_sb = const.tile([P, ND], f32, tag="g")
            nc.sync.dma_start(g_sb[:], dr["ln_g"].ap().rearrange("(o p) -> p o", p=P))
            b_sb = const.tile([P, ND], f32, tag="b")
            nc.sync.dma_start(b_sb[:], dr["ln_b"].ap().rearrange("(o p) -> p o", p=P))
            bout_sb = const.tile([P, 3], f32, tag="bout")
            nc.sync.dma_start(bout_sb[:], dr["bout"].ap().rearrange("(o p) -> p o", p=P))

            ktiles = load_rows("memkT", S, d_rows, [f"t14_{i}" for i in range(ND)], f32)

            from concourse import bass_isa

            # ---------- qT[d, t] = Wqp @ query.T (exact fp32; f32r copy for qh) ----
            # emitted first so the PE has work while the tables normalize
            qTr_tags = ["qry0", "qry1", "qry2", "wqp0"]
            qT = []
            for dt_i in range(ND):
                t_ = main.tile([P, T], f32, tag=f"qt{dt_i}", name=f"q{dt_i}")
                ps = psmm.tile([P, T], f32, tag="mm")
                for c in range(3):
                    nc.tensor.matmul(
                        ps, lhsT=wqpT[c][:, ts(dt_i, P)], rhs=qryT[c][:],
                        start=(c == 0), stop=(c == 2),
                    )
                nc.scalar.copy(t_[:], ps)
                qT.append(t_)
            qTr = []
            for dt_i in range(ND):
                tr_ = main.tile([P, T], f32r, tag=qTr_tags[dt_i], name=f"qr{dt_i}")
                nc.vector.tensor_copy(tr_[:], qT[dt_i][:])
                qTr.append(tr_)

            wqT = load_rows("WqT", D, d_rows, [f"wq{i}" for i in range(ND)], f32r)
            wkT = load_rows("WkT", D, d_rows, [f"wkw{i}" for i in range(ND)], f32r)
            vtiles = load_rows("memvT", S, d_rows, [f"t58_{i}" for i in range(ND)], f32)
            wvT = load_rows("WvT", D, d_rows, [f"wvw{i}" for i in range(ND)], f32r)
            woT = load_rows("WoT", D, d_rows, [f"wo{i}" for i in range(ND)], f32r)
            woutT = load_rows("WoutT", QD, d_rows, [f"wu{i}" for i in range(ND)], f32r)

            # ---------- slot tables: l2-normalize in transposed layout ----------
            # keys (on the scores critical path): PE ones-matmul for the
            # partition sum-of-squares. vals (off critical path): GPSIMD
            # partition_all_reduce, whose output is replicated so the rescale
            # needs no broadcast matmul.
            def normalize_keys(tiles):
                ps_halves = []
                for half in range(2):
                    if half == 0:
                        ps_ssq = psA.tile([1, T], f32, tag="bc", name="ssq0")
                    else:
                        ps_ssq = psA.tile([1, T], f32, tag="ctx", name="ssq1")
                    for i in range(ND):
                        sq = scr2.tile([P, T], f32, tag="sq")
                        nc.scalar.square(sq, tiles[i][:, ds(half * T, T)])
                        nc.tensor.matmul(
                            ps_ssq, lhsT=ones_col, rhs=sq,
                            start=(i == 0), stop=(i == ND - 1),
                        )
                    ps_halves.append(ps_ssq)
                sd_row = main.tile([1, S], f32, tag="sdrow", name="sdr")
                for half in range(2):
                    nc.scalar.activation(
                        sd_row[:, ds(half * T, T)], ps_halves[half], AF.Sqrt,
                        bias=eps_tab[0:1, :],
                    )
                rsq_row = main.tile([1, S], f32, tag="rsqrow", name="rsq")
                nc.vector.reciprocal(rsq_row, sd_row)
                rsqB = main.tile([P, S], f32, tag="rsqB", name="rsqB")
                for half in range(2):
                    ps_b = psA.tile([P, T], f32, tag="bc")
                    nc.tensor.matmul(
                        ps_b, lhsT=ones_row, rhs=rsq_row[:, ds(half * T, T)],
                        start=True, stop=True,
                    )
                    nc.scalar.copy(rsqB[:, ds(half * T, T)], ps_b)
                for i in range(ND):
                    nc.vector.tensor_tensor(tiles[i][:], tiles[i][:], rsqB[:], OP.mult)
                return tiles

            def normalize_vals(tiles):
                sqsum = main.tile([P, S], f32, tag="rsqrow", name="sqs")
                for i in range(ND):
                    sq = main.tile([P, S], f32, tag=f"wk{i}", name=f"vsq{i}")
                    nc.scalar.square(sq[:], tiles[i][:])
                    if i == 0:
                        nc.gpsimd.tensor_copy(sqsum[:], sq[:])
                    else:
                        nc.gpsimd.tensor_tensor(sqsum[:], sqsum[:], sq[:], OP.add)
                rsq_full = main.tile([P, S], f32, tag="rsqB", name="rsqf")
                nc.gpsimd.partition_all_reduce(
                    rsq_full[:], sqsum[:], channels=P, reduce_op=bass_isa.ReduceOp.add
                )
                nc.scalar.activation(sqsum[:], rsq_full[:], AF.Sqrt, bias=eps_tab[:])
                nc.vector.reciprocal(rsq_full[:], sqsum[:])
                for i in range(ND):
                    nc.gpsimd.tensor_tensor(
                        tiles[i][:], tiles[i][:], rsq_full[:], OP.mult
                    )
                return tiles

            # keys; t14 slots chain: keysnT -> mask01
            keysnT = normalize_keys(ktiles)
            # rounded copy of keysnT for the f32r KpT matmul (scores keep fp32)
            ktr = []
            for i in range(ND):
                t_ = main.tile([P, S], f32r, tag=f"ktr{i}", name=f"ktr{i}")
                nc.vector.tensor_copy(t_[:], keysnT[i][:])
                ktr.append(t_)
            # vals; t58 slots chain: valsnT -> scores; wk: vals-sq -> topk scratch
            valsnT = normalize_vals(vtiles)
            vtr_tags = ["sdrow", "rsqrow", "rsqB", "vtr3"]
            vtr = []
            for i in range(ND):
                t_ = main.tile([P, S], f32r, tag=vtr_tags[i], name=f"vtr{i}")
                nc.vector.tensor_copy(t_[:], valsnT[i][:])
                vtr.append(t_)

            # ---------- scores[t, s] = q @ keysn.T (exact fp32), then top-32 ------
            sc = []
            for tt in range(NT):
                t_ = main.tile([P, S], f32, tag=f"t58_{tt}", name=f"sc{tt}")
                for half in range(2):
                    ps = psmm.tile([P, T], f32, tag="mm")
                    for dc in range(ND):
                        nc.tensor.matmul(
                            ps,
                            lhsT=qT[dc][:, ts(tt, P)],
                            rhs=keysnT[dc][:, ds(half * T, T)],
                            start=(dc == 0), stop=(dc == ND - 1),
                        )
                    nc.scalar.copy(t_[:, ds(half * T, T)], ps)
                sc.append(t_)

            # top-32 threshold per token row (4 rounds of max8), then bf16 mask
            mask01 = []
            for tt in range(NT):
                work = main.tile([P, S], f32, tag=f"wk{tt}", name=f"wk{tt}")
                cur = sc[tt]
                for r in range(4):
                    mx = main.tile([P, 8], f32, tag=f"mx{tt}_{r}", name=f"mx{tt}_{r}")
                    nc.vector.max(out=mx[:], in_=cur[:])
                    if r < 3:
                        nc.vector.match_replace(
                            out=work[:], in_to_replace=mx[:], in_values=cur[:],
                            imm_value=NEG,
                        )
                        cur = work
                m_ = main.tile([P, S], f32, tag=f"t14_{tt}", name=f"mk{tt}")
                nc.vector.tensor_scalar(
                    m_[:], sc[tt][:], mx[:, 7:8], None, op0=OP.is_ge
                )
                mask01.append(m_)

            # ---------- KpT[e, s] = Wk @ keysn.T  (f32r) ----------
            kpT = []
            for e in range(ND):
                t_ = main.tile([P, S], f32r, tag=f"kp{e}", name=f"kp{e}")
                for half in range(2):
                    ps = psmm.tile([P, T], f32, tag="mm")
                    for dc in range(ND):
                        nc.tensor.matmul(
                            ps,
                            lhsT=wkT[dc][:, ts(e, P)],
                            rhs=ktr[dc][:, ds(half * T, T)],
                            start=(dc == 0), stop=(dc == ND - 1),
                        )
                    nc.scalar.copy(t_[:, ds(half * T, T)], ps)
                kpT.append(t_)

            # ---------- Vp[s, 8 heads x (64 + ones)] = valsn @ Wv.T (bf16) --------
            vp = []
            for st in range(NS):
                t_ = main.tile([P, H, DH + 1], bf16, tag=f"vp{st}", name=f"vp{st}")
                nc.vector.memset(t_[:, :, DH : DH + 1], 1.0)
                ps = psmm.tile([P, D], f32, tag="mm")
                for dc in range(ND):
                    nc.tensor.matmul(
                        ps,
                        lhsT=vtr[dc][:, ts(st, P)],
                        rhs=wvT[dc][:],
                        start=(dc == 0), stop=(dc == ND - 1),
                    )
                nc.vector.tensor_copy(
                    t_[:, :, 0:DH], ps.rearrange("p (h e) -> p h e", h=H)
                )
                vp.append(t_)

            # ---------- qhT[e, t] = (Wq @ qT) / 8  (f32r) ----------
            qhT = []
            for e in range(ND):
                t_ = main.tile([P, T], f32r, tag=f"wvw{e}", name=f"qh{e}")
                ps = psmm.tile([P, T], f32, tag="mm")
                for dc in range(ND):
                    nc.tensor.matmul(
                        ps, lhsT=wqT[dc][:, ts(e, P)], rhs=qTr[dc][:],
                        start=(dc == 0), stop=(dc == ND - 1),
                    )
                nc.scalar.mul(t_[:], ps, 1.0 / np.sqrt(DH))
                qhT.append(t_)

            # ---------- masked attention over all S slots ----------
            # u (exp output) rotates over 20 dead slots: 4 retired qT tiles
            # plus 4x4 quarter-slices of the retired ktr tiles (free after the
            # KpT matmuls, i.e. BEFORE the top-k finishes). The first 20
            # units' logit matmuls + exps are EMITTED BEFORE the mask
            # transposes: they don't read the masks, so the PE/ACT stream
            # keeps working while the DVE finishes the top-k (a stalled
            # transpose would otherwise block everything behind it in PE
            # program order).
            u_singles = [
                main.tile([P, T], bf16, tag=f"qt{i}", name=f"us{i}")
                for i in range(4)
            ]
            u_quads = [
                main.tile([P, 4, T], bf16, tag=f"ktr{i}", name=f"uq{i}")
                for i in range(4)
            ]

            def u_slot(unit):
                m = unit % 20
                if m < 4:
                    return u_singles[m][:]
                m -= 4
                return u_quads[m // 4][:, m % 4, :]

            def att_exp(unit):
                h, c = unit // NS, unit % NS
                et, ro = h // 2, (h % 2) * 64
                ps_att = psmm.tile([P, T], f32, tag="mm", name=f"att{unit}")
                nc.tensor.matmul(
                    ps_att,
                    lhsT=kpT[et][ro : ro + DH, ts(c, P)],
                    rhs=qhT[et][ro : ro + DH, :],
                    start=True, stop=True,
                )
                u = u_slot(unit)
                nc.scalar.activation(u[:], ps_att, AF.Exp)
                return u

            PRE = 8
            u_pre = {unit: att_exp(unit) for unit in range(PRE)}

            # ---------- transpose the mask to [s, t] (bf16 PE transposes) ---------
            mT = []
            for j in range(NS):
                tag = ["qry0", "qry1", "qry2", "wqp0", "mT4", "mT5", "mT6", "mT7"][j]
                mT.append(main.tile([P, T], bf16, tag=tag, name=f"mT{j}"))
            for j in range(NS):
                ps_t = psA.tile([P, T], f32, tag="bc", name=f"pst{j}")
                for tt in range(NT):
                    nc.tensor.matmul(
                        ps_t[:, ts(tt, P)], lhsT=mask01[tt][:, ts(j, P)],
                        rhs=ident_f, is_transpose=True, skip_group_check=True,
                    )
                nc.scalar.copy(mT[j][:], ps_t)

            # wkw slots chain: WkT -> ctxT
            ctxT = [
                main.tile([P, T], f32, tag=f"wkw{dt_i}", name=f"cx{dt_i}")
                for dt_i in range(ND)
            ]
            for h in range(H):
                et, ro = h // 2, (h % 2) * 64
                if h % 2 == 0:
                    den_pair = scr2.tile([1, 2 * T], f32r, tag="den")
                ps_ctx = psA.tile([DH + 1, T], f32, tag="ctx")
                for c in range(NS):
                    unit = h * NS + c
                    u = u_pre.pop(unit) if unit in u_pre else att_exp(unit)
                    w = scr4.tile([P, T], bf16, tag="w")
                    nc.vector.tensor_tensor(w[:], u[:], mT[c][:], OP.mult)
                    nc.tensor.matmul(
                        ps_ctx, lhsT=vp[c][:, h, :], rhs=w[:],
                        start=(c == 0), stop=(c == NS - 1),
                    )
                nc.vector.tensor_copy(
                    ctxT[et][ro : ro + DH, :].bitcast(f32r), ps_ctx[0:DH, :]
                )
                # reciprocal straight from the PSUM denominator row — no copy
                nc.vector.reciprocal(
                    den_pair[0:1, ds((h % 2) * T, T)], ps_ctx[DH : DH + 1, :]
                )
                if h % 2 == 1:
                    # divide the head pair's ctx rows by their softmax denominators
                    ps_rb = psA.tile([P, T], f32, tag="bc")
                    nc.tensor.matmul(
                        ps_rb, lhsT=selA, rhs=den_pair[0:1, 0:T],
                        start=True, stop=False,
                    )
                    nc.tensor.matmul(
                        ps_rb, lhsT=selB, rhs=den_pair[0:1, T : 2 * T],
                        start=False, stop=True,
                    )
                    nc.vector.tensor_tensor(
                        ctxT[et][:].bitcast(f32r), ctxT[et][:], ps_rb, OP.mult
                    )

            # ---------- oT[e, t] = Wo @ ctx.T  (f32r); wq slots -> oT ----------
            oT = []
            for e in range(ND):
                t_ = main.tile([P, T], f32, tag=f"wq{e}", name=f"o{e}")
                ps = psmm.tile([P, T], f32, tag="mm")
                for dc in range(ND):
                    nc.tensor.matmul(
                        ps, lhsT=woT[dc][:, ts(e, P)],
                        rhs=ctxT[dc][:].bitcast(f32r),
                        start=(dc == 0), stop=(dc == ND - 1),
                    )
                nc.scalar.copy(t_[:], ps)
                oT.append(t_)

            # ---------- LayerNorm over e (partitions), stats via ones-matmul -----
            ps_mu = psA.tile([1, T], f32, tag="bc", name="psmu")
            ps_ms = psA.tile([1, T], f32, tag="ctx", name="psms")
            for dc in range(ND):
                sq = scr2.tile([P, T], f32, tag="sq")
                nc.scalar.square(sq, oT[dc][:])
                nc.tensor.matmul(
                    ps_mu, lhsT=ones_col, rhs=oT[dc][:],
                    start=(dc == 0), stop=(dc == ND - 1),
                )
                nc.tensor.matmul(
                    ps_ms, lhsT=ones_col, rhs=sq[:],
                    start=(dc == 0), stop=(dc == ND - 1),
                )
            mu_row = main.tile([1, T], f32, tag="mu", name="mu")
            ms_row = main.tile([1, T], f32, tag="ms", name="ms")
            nc.scalar.mul(mu_row[:], ps_mu, 1.0 / D)
            nc.scalar.mul(ms_row[:], ps_ms, 1.0 / D)
            var_row = main.tile([1, T], f32, tag="var", name="var")
            nc.vector.tensor_tensor(var_row[:], mu_row[:], mu_row[:], OP.mult)
            nc.vector.tensor_sub(var_row[:], ms_row[:], var_row[:])
            sd_row2 = main.tile([1, T], f32, tag="sd", name="sd2")
            nc.scalar.activation(sd_row2[:], var_row[:], AF.Sqrt, bias=eps_ln[:])
            rstd_row = main.tile([1, T], f32, tag="rstd", name="rstd")
            nc.vector.reciprocal(rstd_row[:], sd_row2[:])
            crow_r = main.tile([1, T], f32r, tag="mu2", name="crow_r")
            nc.vector.scalar_tensor_tensor(
                crow_r[:], mu_row[:], -1.0, rstd_row[:], op0=OP.mult, op1=OP.mult
            )
            rstd_r = main.tile([1, T], f32r, tag="ms2", name="rstd_r")
            nc.vector.tensor_copy(rstd_r[:], rstd_row[:])
            bcasts = []
            for row in (rstd_r, crow_r):
                ps_b = psA.tile([P, T], f32, tag="bc", name=f"lnb{len(bcasts)}")
                nc.tensor.matmul(
                    ps_b, lhsT=ones_row_r, rhs=row[:], start=True, stop=True
                )
                bcasts.append(ps_b)
            rstdB, cB = bcasts
            nrm = []
            for dt_i in range(ND):
                nc.vector.tensor_tensor(oT[dt_i][:], oT[dt_i][:], rstdB, OP.mult)
                nc.vector.tensor_tensor(oT[dt_i][:], oT[dt_i][:], cB, OP.add)
                n_ = main.tile([P, T], f32r, tag=f"wkw{dt_i}", name=f"nrm{dt_i}")
                nc.vector.scalar_tensor_tensor(
                    n_[:], oT[dt_i][:], g_sb[:, dt_i : dt_i + 1],
                    b_sb[:, dt_i : dt_i + 1].to_broadcast([P, T]),
                    op0=OP.mult, op1=OP.add,
                )
                nrm.append(n_)

            # ---------- outT[q, t] = Wout @ normed.T + bout ----------
            for qt, (off, sz) in enumerate(QD_TILES):
                ps = psmm.tile([P, T], f32, tag="mm")
                for e in range(ND):
                    nc.tensor.matmul(
                        ps[:sz, :], lhsT=woutT[e][:, ds(off, sz)], rhs=nrm[e][:],
                        start=(e == 0), stop=(e == ND - 1),
                    )
                ot_sb = scr2.tile([P, T], f32, tag="ot")
                nc.scalar.add(ot_sb[:sz, :], ps[:sz, :], bout_sb[:sz, qt : qt + 1])
                nc.sync.dma_start(out_dram.ap()[ds(off, sz), :], ot_sb[:sz, :])

    nc.compile()
    return nc


def _prep_in_maps(inputs):
    def c(a):
        return np.ascontiguousarray(a, dtype=np.float32)

    q = np.asarray(inputs["query_states"], dtype=np.float32).reshape(B * N, QD)
    shared = {
        "WqpT": c(np.asarray(inputs["Wqp"]).T),
        "WqT": c(np.asarray(inputs["Wq"]).T),
        "WkT": c(np.asarray(inputs["Wk"]).T),
        "WvT": c(np.asarray(inputs["Wv"]).T),
        "WoT": c(np.asarray(inputs["Wo"]).T),
        "WoutT": c(np.asarray(inputs["Wout"]).T),
        "memkT": c(np.asarray(inputs["mem_keys"]).T),
        "memvT": c(np.asarray(inputs["mem_values"]).T),
        "ln_g": c(np.asarray(inputs["ln_g"])),
        "ln_b": c(np.asarray(inputs["ln_b"])),
        "bout": c(np.pad(np.asarray(inputs["bout"]), (0, 384 - QD))),
    }
    in_maps = []
    for core in range(NCORES):
        m = dict(shared)
        m["queryT"] = c(q[core * T : (core + 1) * T, :].T)
        in_maps.append(m)
    return in_maps


def kernel(**inputs) -> np.ndarray:
    if "nc" not in _CACHE:
        _CACHE["nc"] = _build_nc()
    nc = _CACHE["nc"]
    in_maps = _prep_in_maps(inputs)
    res = run_bass_kernel_spmd(nc, in_maps, core_ids=list(range(NCORES)))
    out = np.empty((B * N, QD), dtype=np.float32)
    for core in range(NCORES):
        out[core * T : (core + 1) * T, :] = res.results[core]["outT"].T
    return out.reshape(B, N, QD)



# revision 13
# speedup vs baseline: 1.0703x; 1.0703x over previous
"""GatedLTMMemory kernel for 8 Trainium2 NeuronCores.

Data-parallel over the 4096 flattened (B,N) tokens: 512 tokens per core.
Memory-slot tables and weights are replicated. Per-selected-slot projections
are replaced by projecting the slot tables once and running a masked
full-softmax over all S slots (exactly equivalent math).

v2 restructure vs the first working kernel (183.9us):
  - slot-table l2 normalization is OFF the critical path: raw scores
    q.k are computed straight after the DMAs land; the 1/|k| column scale
    is applied on the PSUM->SBUF copy (DVE). Norms come from ACT squares +
    Pool adds + GPSIMD partition_all_reduce (output replicated across
    partitions, so no PE broadcast matmul is needed).
  - single-descriptor DMA loads ((a p) d -> p (a d) rearrange) cut HWDGE
    descriptor serialization from ~25us to ~9us.
  - attention is split into two 256-token halves; within a half, 4 slot
    chunks share one [128,1024] PSUM quad -> one 1024-wide exp. The
    epilogue (Wo+LN+Wout) of half 0 runs under the attention of half 1.
  - LayerNorm stats, Wo/Wout, attention all f32r; fp32 only where the
    top-32 selection needs it (qT, scores, key norms).
  - f32r operands are bitcast views of fp32 tiles (no convert copies).
"""

import numpy as np

import concourse.bacc as bacc
import concourse.bass_isa as bass_isa
import concourse.mybir as mybir
import concourse.tile as tile
from concourse.bass import ds, ts
from concourse.bass_utils import run_bass_kernel_spmd
from concourse.masks import make_identity

B, N, QD, D, S, H, K = 4, 1024, 320, 512, 1024, 8, 32
DH = D // H
EPS = 1e-5
P = 128
T = 512                       # tokens per core
HT = 256                      # tokens per epilogue half
NCORES = 8
NT = T // P                   # 4 token tiles
ND = D // P                   # 4 contraction chunks over D
NS = S // P                   # 8 slot chunks
NEG = -1e30
QD_TILES = [(0, 128), (128, 128), (256, 64)]

f32 = mybir.dt.float32
f32r = mybir.dt.float32r
bf16 = mybir.dt.bfloat16
AF = mybir.ActivationFunctionType
OP = mybir.AluOpType

_CACHE: dict = {}


def _build_nc():
    nc = bacc.Bacc("TRN2", target_bir_lowering=False, debug=False)

    dr = {}

    def din(name, shape, dt_):
        dr[name] = nc.dram_tensor(name, shape, dt_, kind="ExternalInput")

    din("queryT", (QD, T), f32)
    din("WqpT", (QD, D), f32)
    din("WqT", (D, D), f32r)
    din("WkT", (D, D), f32r)
    din("WvT", (D, D), f32r)
    din("WoT", (D, D), f32r)
    din("WoutT", (D, QD), f32r)
    din("memkT", (D, S), f32)
    din("memkTr", (D, S), f32r)
    din("memvT", (D, S), f32r)
    din("ln_g", (D,), f32)
    din("ln_b", (D,), f32)
    din("bout", (384,), f32)
    out_dram = nc.dram_tensor("outT", (QD, T), f32, kind="ExternalOutput")

    with tile.TileContext(nc) as tc:
        with (
            tc.tile_pool(name="const", bufs=1) as const,
            tc.tile_pool(name="main", bufs=1) as main,
            tc.tile_pool(name="scr2", bufs=2) as scr2,
            tc.tile_pool(name="wpool", bufs=4) as wpool,
            tc.tile_pool(name="psmm", bufs=2, space="PSUM") as psmm,
            tc.tile_pool(name="psq", bufs=2, space="PSUM") as psq,
            tc.tile_pool(name="psctx", bufs=1, space="PSUM") as psctx,
            tc.tile_pool(name="psaux", bufs=1, space="PSUM") as psaux,
            nc.allow_low_precision(reason="validated f32r/bf16 paths"),
        ):
            # ---------- constants ----------
            ident = const.tile([P, P], bf16, tag="ident")
            make_identity(nc, ident)
            ones_col = const.tile([P, 1], f32, tag="ones_col")
            nc.vector.memset(ones_col, 1.0)
            ones_row = const.tile([1, P], f32, tag="ones_row")
            nc.vector.memset(ones_row, 1.0)
            # selA/selB rows for per-head-pair denominator broadcast
            halfsel = const.tile([1, 2 * P], f32, tag="halfsel")
            nc.vector.memset(halfsel, 0.0)
            nc.vector.memset(halfsel[0:1, 64:192], 1.0)
            halfsel_r = const.tile([1, 2 * P], f32r, tag="halfsel_r")
            nc.scalar.copy(halfsel_r[:], halfsel[:])
            # layout: [0:64]=0, [64:192]=1, [192:256]=0
            ones_row_r = halfsel_r[0:1, 64:192]  # [1,128] ones
            selA = halfsel_r[0:1, 128:256]       # ones x64, zeros x64
            selB = halfsel_r[0:1, 0:128]         # zeros x64, ones x64
            ones_col_r = const.tile([P, 1], f32r, tag="ones_col_r")
            nc.scalar.copy(ones_col_r[:], ones_col[:])
            eps_tab = const.tile([P, 1], f32, tag="eps_tab")
            nc.vector.memset(eps_tab, 1e-12)
            eps_ln = const.tile([1, 1], f32, tag="eps_ln")
            nc.vector.memset(eps_ln, EPS)

            # ---------- DMA loads (critical tensors first) ----------
            def load_qd_tiles(name, cols, tags, dt_):
                tiles = []
                for (off, sz), tag in zip(QD_TILES, tags):
                    t_ = main.tile([sz, cols], dt_, tag=tag, name=f"ld_{tag}")
                    nc.sync.dma_start(t_[:], dr[name].ap()[ds(off, sz), :])
                    tiles.append(t_)
                return tiles

            def load_wide(name, inner, dt_, tag):
                # (a p) s -> p (a s): one descriptor for a (ND*P, inner) tensor
                t_ = main.tile([P, ND, inner], dt_, tag=tag, name=f"ld_{tag}")
                nc.sync.dma_start(
                    t_[:], dr[name].ap().rearrange("(a p) s -> p a s", p=P)
                )
                return t_

            qryT = load_qd_tiles("queryT", T, ["qry0", "qry1", "qry2"], f32)
            wqpT = load_qd_tiles("WqpT", D, ["wqp0", "wqp1", "wqp2"], f32)
            ktab = load_wide("memkT", S, f32, "ktab")       # [128, dc, S]
            wq = load_wide("WqT", D, f32r, "wq")            # [128, dc, D]
            wk = load_wide("WkT", D, f32r, "wk")
            vtab = load_wide("memvT", S, f32r, "vtab")
            ktabr = load_wide("memkTr", S, f32r, "ktabr")
            wv = load_wide("WvT", D, f32r, "wv")
            wo = load_wide("WoT", D, f32r, "wo")
            wout = load_wide("WoutT", QD, f32r, "wout")     # [128, dc, QD]

            g_sb = const.tile([P, ND], f32, tag="g")
            nc.sync.dma_start(g_sb[:], dr["ln_g"].ap().rearrange("(o p) -> p o", p=P))
            b_sb = const.tile([P, ND], f32, tag="b")
            nc.sync.dma_start(b_sb[:], dr["ln_b"].ap().rearrange("(o p) -> p o", p=P))
            bout_sb = const.tile([P, 3], f32, tag="bout")
            nc.sync.dma_start(bout_sb[:], dr["bout"].ap().rearrange("(o p) -> p o", p=P))

            # ---------- qT[d, t] = Wqp @ query.T (exact fp32) ----------
            qT = []
            for dt_i in range(ND):
                t_ = main.tile([P, T], f32, tag=f"qt{dt_i}", name=f"q{dt_i}")
                ps = psmm.tile([P, T], f32, tag="mm")
                for c in range(3):
                    nc.tensor.matmul(
                        ps, lhsT=wqpT[c][:, ts(dt_i, P)], rhs=qryT[c][:],
                        start=(c == 0), stop=(c == 2),
                    )
                nc.scalar.copy(t_[:], ps)
                qT.append(t_)
            qTr = []
            for dt_i in range(ND):
                tr_ = main.tile([P, T], f32r, tag=f"qtr{dt_i}", name=f"qr{dt_i}")
                nc.vector.tensor_copy(tr_[:], qT[dt_i][:])
                qTr.append(tr_)

            # ---------- key inverse-norms (replicated), off the PE ----------
            # rsqB[p, s] = 1/sqrt(sum_d k[d,s]^2 + 1e-12), same value on all
            # partitions p (partition_all_reduce replicates its output).
            def inv_norms(tab, acc_tag, tag_out):
                # square+sum in [P, T]-column halves to keep scratch small
                acc = main.tile([P, S], f32, tag=acc_tag, name=f"{tag_out}_acc")
                for half in range(2):
                    col = ds(half * T, T)
                    for i in range(ND):
                        src_ap = tab[:, i, col]
                        if src_ap.dtype != f32:
                            src_ap = src_ap.bitcast(f32)
                        if i == 0:
                            nc.scalar.square(acc[:, col], src_ap)
                        else:
                            sq0 = scr2.tile([P, T], f32, tag="sq", name=f"{tag_out}_sq")
                            nc.scalar.square(sq0[:], src_ap)
                            nc.gpsimd.tensor_tensor(acc[:, col], acc[:, col], sq0[:], OP.add)
                red = main.tile([P, S], f32, tag=tag_out, name=tag_out)
                nc.gpsimd.partition_all_reduce(
                    red[:], acc[:], channels=P, reduce_op=bass_isa.ReduceOp.add
                )
                # sqrt(x + eps) on ACT, reciprocal on DVE (Rsqrt is blocked)
                nc.scalar.activation(red[:], red[:], AF.Sqrt, bias=eps_tab[:])
                nc.vector.reciprocal(red[:], red[:])
                return red

            rsqB = inv_norms(ktab, "kp0", "rsqB")

            # ---------- scores[t, s] = (q @ k_raw.T) * rsq  (exact fp32) ------
            sc = []
            for tt in range(NT):
                t_ = main.tile([P, S], f32, tag=f"sc{tt}", name=f"sc{tt}")
                for half in range(2):
                    ps = psmm.tile([P, T], f32, tag="mm")
                    for dc in range(ND):
                        nc.tensor.matmul(
                            ps,
                            lhsT=qT[dc][:, ts(tt, P)],
                            rhs=ktab[:, dc, ds(half * T, T)],
                            start=(dc == 0), stop=(dc == ND - 1),
                        )
                    # scale-on-copy: normalized scores land in SBUF
                    nc.vector.tensor_tensor(
                        t_[:, ds(half * T, T)], ps, rsqB[:, ds(half * T, T)],
                        OP.mult,
                    )
                sc.append(t_)

                # top-32 threshold per token row (4 rounds of max8), 0/1 mask
                work = main.tile([P, S], f32, tag=f"wk{tt % 2}", name=f"wk{tt}")
                cur = t_
                for r in range(4):
                    mx = main.tile([P, 8], f32, tag=f"mx{tt}_{r}", name=f"mx{tt}_{r}")
                    nc.vector.max(out=mx[:], in_=cur[:])
                    if r < 3:
                        nc.vector.match_replace(
                            out=work[:], in_to_replace=mx[:], in_values=cur[:],
                            imm_value=NEG,
                        )
                        cur = work
                mk_tags = ["qry0", "qry1", "wqp0", "wqp1"]
                m_ = main.tile([P, S], bf16, tag=mk_tags[tt], name=f"mk{tt}")
                nc.vector.tensor_scalar(
                    m_[:], t_[:], mx[:, 7:8], None, op0=OP.is_ge
                )
                sc.append(m_)
            mask01 = [sc[2 * tt + 1] for tt in range(NT)]

            # ---------- value inverse-norms (Pool, off critical path) ----
            rsvB = inv_norms(vtab, "kp1", "rsvB")

            # ---------- KpT[e, s] = Wk @ k_raw.T (f32r), scaled in-place -----
            kp = []
            for e in range(ND):
                t_ = main.tile([P, S], f32r, tag=f"kp{e}", name=f"kp{e}")
                for half in range(2):
                    ps = psmm.tile([P, T], f32, tag="mm")
                    for dc in range(ND):
                        nc.tensor.matmul(
                            ps,
                            lhsT=wk[:, dc, ts(e, P)],
                            rhs=ktabr[:, dc, ds(half * T, T)],
                            start=(dc == 0), stop=(dc == ND - 1),
                        )
                    nc.scalar.copy(t_[:, ds(half * T, T)], ps)
                kp.append(t_)

            # rsv in [slot-partition, 1] layout per chunk: 8 tiny PE transposes
            ps_rsv = psaux.tile([P, 8], f32, tag="aux", name="ps_rsv")
            for st in range(NS):
                nc.tensor.matmul(
                    ps_rsv[:, st : st + 1],
                    lhsT=rsvB[0:1, ts(st, P)], rhs=ones_row[0:1, 0:1],
                    is_transpose=True, skip_group_check=True,
                )
            rsv_sb = const.tile([P, 8], f32, tag="rsv_sb")
            nc.vector.tensor_copy(rsv_sb[:], ps_rsv)

            # ---------- Vp[s, 8 heads x (64 + ones)] = valsn @ Wv.T (bf16) ----
            vp = []
            for st in range(NS):
                t_ = main.tile([P, H, DH + 1], bf16, tag=f"vp{st}", name=f"vp{st}")
                nc.vector.memset(t_[:, :, DH : DH + 1], 1.0)
                ps = psmm.tile([P, D], f32, tag="mm")
                for dc in range(ND):
                    nc.tensor.matmul(
                        ps,
                        lhsT=vtab[:, dc, ts(st, P)],
                        rhs=wv[:, dc, :],
                        start=(dc == 0), stop=(dc == ND - 1),
                    )
                nc.scalar.activation(
                    t_[:, :, 0:DH], ps.rearrange("p (h e) -> p h e", h=H),
                    AF.Copy, scale=rsv_sb[:, st : st + 1],
                )
                vp.append(t_)

            # ---------- qhT[e, t] = (Wq @ qT) / 8  (f32r) ----------
            qh = []
            for e in range(ND):
                t_ = main.tile([P, T], f32r, tag=f"qh{e}", name=f"qh{e}")
                ps = psmm.tile([P, T], f32, tag="mm")
                for dc in range(ND):
                    nc.tensor.matmul(
                        ps, lhsT=wq[:, dc, ts(e, P)], rhs=qTr[dc][:],
                        start=(dc == 0), stop=(dc == ND - 1),
                    )
                nc.scalar.mul(t_[:], ps, 1.0 / np.sqrt(DH))
                qh.append(t_)

            # kp column scale by 1/|k_s| (DVE, after top-k drains)
            for e in range(ND):
                nc.vector.tensor_tensor(
                    kp[e][:], kp[e][:].bitcast(f32), rsqB[:], OP.mult
                )

            # ---------- transpose masks to [s, t] (bf16 PE transposes) -------
            # mTq[g][:, i, :] = mask chunk (4g+i) transposed, full T columns
            mTq = []
            for g in range(2):
                mt = main.tile([P, 4, T], bf16, tag=f"sc{g}", name=f"mTq{g}")
                for i in range(4):
                    ps_t = psaux.tile([P, T], bf16, tag="aux", name=f"pst{g}{i}")
                    for tt in range(NT):
                        nc.tensor.matmul(
                            ps_t[:, ts(tt, P)],
                            lhsT=mask01[tt][:, ts(4 * g + i, P)],
                            rhs=ident, is_transpose=True, skip_group_check=True,
                        )
                    nc.scalar.copy(mt[:, i, :], ps_t)
                mTq.append(mt)

            # ---------- attention: per 256-token half, quads of 4 chunks -----
            # ctxT[et][ro:ro+DH, t] per head; epilogue per half underneath the
            # other half's attention.
            cx_tags = ["sc2", "sc3", "wk0", "wk1"]
            ctxT = [
                main.tile([P, T], f32, tag=cx_tags[dt_i], name=f"cx{dt_i}")
                for dt_i in range(ND)
            ]
            oT_big = main.tile([P, ND, T], f32, tag="vtab", name="oT")
            oT = [oT_big[:, dt_i, :] for dt_i in range(ND)]

            def attention_half(half):
                tok = ds(half * HT, HT)
                for h in range(H):
                    et, ro = h // 2, (h % 2) * 64
                    if h % 2 == 0:
                        den_pair = scr2.tile([1, 2 * HT], f32r, tag="den")
                    ps_ctx = psctx.tile([DH + 1, HT], f32, tag="ctx")
                    for g in range(2):
                        ps_att = psq.tile([P, 4, HT], f32, tag="q")
                        for i in range(4):
                            nc.tensor.matmul(
                                ps_att[:, i, :],
                                lhsT=kp[et][ro : ro + DH, ts(4 * g + i, P)],
                                rhs=qh[et][ro : ro + DH, tok],
                                start=True, stop=True, skip_group_check=True,
                            )
                        w = scr2.tile([P, 4, HT], bf16, tag="u")
                        nc.scalar.activation(w[:], ps_att, AF.Exp)
                        nc.vector.tensor_tensor(
                            w[:], w[:], mTq[g][:, :, tok], OP.mult
                        )
                        for i in range(4):
                            nc.tensor.matmul(
                                ps_ctx, lhsT=vp[4 * g + i][:, h, :], rhs=w[:, i, :],
                                start=(g == 0 and i == 0), stop=(g == 1 and i == 3),
                            )
                    nc.vector.tensor_copy(
                        ctxT[et][ro : ro + DH, tok].bitcast(f32r), ps_ctx[0:DH, :]
                    )
                    nc.vector.reciprocal(
                        den_pair[0:1, ds((h % 2) * HT, HT)], ps_ctx[DH : DH + 1, :]
                    )
                    if h % 2 == 1:
                        ps_rb = psaux.tile([P, HT], f32, tag="aux")
                        nc.tensor.matmul(
                            ps_rb, lhsT=selA, rhs=den_pair[0:1, 0:HT],
                            start=True, stop=False,
                        )
                        nc.tensor.matmul(
                            ps_rb, lhsT=selB, rhs=den_pair[0:1, HT : 2 * HT],
                            start=False, stop=True,
                        )
                        nc.vector.tensor_tensor(
                            ctxT[et][:, tok].bitcast(f32r), ctxT[et][:, tok],
                            ps_rb, OP.mult,
                        )

            def epilogue_half(half):
                tok = ds(half * HT, HT)
                # oT[e, t] = Wo @ ctx.T (f32r)
                for e in range(ND):
                    ps = psmm.tile([P, T], f32, tag="mm")
                    for dc in range(ND):
                        nc.tensor.matmul(
                            ps[:, 0:HT], lhsT=wo[:, dc, ts(e, P)],
                            rhs=ctxT[dc][:, tok].bitcast(f32r),
                            start=(dc == 0), stop=(dc == ND - 1),
                        )
                    nc.scalar.copy(oT[e][:, tok].bitcast(f32r), ps[:, 0:HT])
                # LayerNorm stats via f32r ones-matmuls
                ps_mu = psaux.tile([P, T], f32, tag="aux")
                sqs = []
                for dc in range(ND):
                    sq = scr2.tile([P, HT], f32r, tag="lnsq")
                    nc.scalar.square(sq, oT[dc][:, tok])
                    sqs.append(sq)
                for dc in range(ND):
                    nc.tensor.matmul(
                        ps_mu[0:1, 0:HT], lhsT=ones_col_r[:],
                        rhs=oT[dc][:, tok].bitcast(f32r),
                        start=(dc == 0), stop=(dc == ND - 1),
                        skip_group_check=True,
                    )
                for dc in range(ND):
                    nc.tensor.matmul(
                        ps_mu[0:1, HT : 2 * HT], lhsT=ones_col_r[:],
                        rhs=sqs[dc][:],
                        start=(dc == 0), stop=(dc == ND - 1),
                        skip_group_check=True,
                    )
                mu_row = main.tile([1, HT], f32, tag="mu", name=f"mu{half}")
                ms_row = main.tile([1, HT], f32, tag="ms", name=f"ms{half}")
                nc.scalar.mul(mu_row[:], ps_mu[0:1, 0:HT], 1.0 / D)
                nc.scalar.mul(ms_row[:], ps_mu[0:1, HT : 2 * HT], 1.0 / D)
                var_row = main.tile([1, HT], f32, tag="var", name=f"var{half}")
                nc.vector.tensor_tensor(var_row[:], mu_row[:], mu_row[:], OP.mult)
                nc.vector.tensor_sub(var_row[:], ms_row[:], var_row[:])
                sd_row = main.tile([1, HT], f32, tag="sd", name=f"sd{half}")
                nc.scalar.activation(sd_row[:], var_row[:], AF.Sqrt, bias=eps_ln[:])
                rstd_row = main.tile([1, HT], f32r, tag="rstd", name=f"rstd{half}")
                nc.vector.reciprocal(rstd_row[:], sd_row[:])
                crow = main.tile([1, HT], f32r, tag="crow", name=f"crow{half}")
                nc.vector.scalar_tensor_tensor(
                    crow[:], mu_row[:], -1.0, rstd_row[:].bitcast(f32),
                    op0=OP.mult, op1=OP.mult
                )
                ps_b = psaux.tile([P, T], f32, tag="aux")
                nc.tensor.matmul(
                    ps_b[:, 0:HT], lhsT=ones_row_r, rhs=rstd_row[:],
                    start=True, stop=True, skip_group_check=True,
                )
                nc.tensor.matmul(
                    ps_b[:, HT : 2 * HT], lhsT=ones_row_r, rhs=crow[:],
                    start=True, stop=True, skip_group_check=True,
                )
                nrm = []
                for dt_i in range(ND):
                    nc.vector.tensor_tensor(
                        oT[dt_i][:, tok].bitcast(f32r), oT[dt_i][:, tok],
                        ps_b[:, 0:HT], OP.mult,
                    )
                    nc.vector.tensor_tensor(
                        oT[dt_i][:, tok].bitcast(f32r), oT[dt_i][:, tok],
                        ps_b[:, HT : 2 * HT], OP.add,
                    )
                    n_ = scr2.tile([P, HT], f32r, tag=f"nrm{dt_i % 2}")
                    nc.vector.scalar_tensor_tensor(
                        n_[:], oT[dt_i][:, tok], g_sb[:, dt_i : dt_i + 1],
                        b_sb[:, dt_i : dt_i + 1].to_broadcast([P, HT]),
                        op0=OP.mult, op1=OP.add,
                    )
                    nrm.append(n_)
                for qt, (off, sz) in enumerate(QD_TILES):
                    ps = psmm.tile([P, T], f32, tag="mm")
                    for e in range(ND):
                        nc.tensor.matmul(
                            ps[:sz, 0:HT], lhsT=wout[:, e, ds(off, sz)],
                            rhs=nrm[e][:],
                            start=(e == 0), stop=(e == ND - 1),
                        )
                    ot_sb = scr2.tile([P, HT], f32, tag="ot")
                    nc.scalar.add(ot_sb[:sz, :], ps[:sz, 0:HT], bout_sb[:sz, qt : qt + 1])
                    nc.sync.dma_start(
                        out_dram.ap()[ds(off, sz), ds(half * HT, HT)], ot_sb[:sz, :]
                    )

            attention_half(0)
            epilogue_half(0)
            attention_half(1)
            epilogue_half(1)

    nc.compile()
    return nc


def _prep_in_maps(inputs):
    def c(a):
        return np.ascontiguousarray(a, dtype=np.float32)

    q = np.asarray(inputs["query_states"], dtype=np.float32).reshape(B * N, QD)
    shared = {
        "WqpT": c(np.asarray(inputs["Wqp"]).T),
        "WqT": c(np.asarray(inputs["Wq"]).T),
        "WkT": c(np.asarray(inputs["Wk"]).T),
        "WvT": c(np.asarray(inputs["Wv"]).T),
        "WoT": c(np.asarray(inputs["Wo"]).T),
        "WoutT": c(np.asarray(inputs["Wout"]).T),
        "memkT": c(np.asarray(inputs["mem_keys"]).T),
        "memkTr": c(np.asarray(inputs["mem_keys"]).T),
        "memvT": c(np.asarray(inputs["mem_values"]).T),
        "ln_g": c(np.asarray(inputs["ln_g"])),
        "ln_b": c(np.asarray(inputs["ln_b"])),
        "bout": c(np.pad(np.asarray(inputs["bout"]), (0, 384 - QD))),
    }
    in_maps = []
    for core in range(NCORES):
        m = dict(shared)
        m["queryT"] = c(q[core * T : (core + 1) * T, :].T)
        in_maps.append(m)
    return in_maps


def kernel(**inputs) -> np.ndarray:
    if "nc" not in _CACHE:
        _CACHE["nc"] = _build_nc()
    nc = _CACHE["nc"]
    in_maps = _prep_in_maps(inputs)
    res = run_bass_kernel_spmd(nc, in_maps, core_ids=list(range(NCORES)))
    out = np.empty((B * N, QD), dtype=np.float32)
    for core in range(NCORES):
        out[core * T : (core + 1) * T, :] = res.results[core]["outT"].T
    return out.reshape(B, N, QD)


# revision 16
# speedup vs baseline: 1.1441x; 1.0690x over previous
"""GatedLTMMemory kernel for 8 Trainium2 NeuronCores.

Data-parallel over the 4096 flattened (B,N) tokens: 512 tokens per core.
Memory-slot tables and weights are replicated. Per-selected-slot projections
are replaced by projecting the slot tables once and running a masked
full-softmax over all S slots (exactly equivalent math).

v2 restructure vs the first working kernel (183.9us):
  - slot-table l2 normalization is OFF the critical path: raw scores
    q.k are computed straight after the DMAs land; the 1/|k| column scale
    is applied on the PSUM->SBUF copy (DVE). Norms come from ACT squares +
    Pool adds + GPSIMD partition_all_reduce (output replicated across
    partitions, so no PE broadcast matmul is needed).
  - single-descriptor DMA loads ((a p) d -> p (a d) rearrange) cut HWDGE
    descriptor serialization from ~25us to ~9us.
  - attention is split into two 256-token halves; within a half, 4 slot
    chunks share one [128,1024] PSUM quad -> one 1024-wide exp. The
    epilogue (Wo+LN+Wout) of half 0 runs under the attention of half 1.
  - LayerNorm stats, Wo/Wout, attention all f32r; fp32 only where the
    top-32 selection needs it (qT, scores, key norms).
  - f32r operands are bitcast views of fp32 tiles (no convert copies).
"""

import numpy as np

import concourse.bacc as bacc
import concourse.bass_isa as bass_isa
import concourse.mybir as mybir
import concourse.tile as tile
from concourse.bass import ds, ts
from concourse.bass_utils import run_bass_kernel_spmd
from concourse.masks import make_identity

B, N, QD, D, S, H, K = 4, 1024, 320, 512, 1024, 8, 32
DH = D // H
EPS = 1e-5
P = 128
T = 512                       # tokens per core
HT = 256                      # tokens per epilogue half
NCORES = 8
NT = T // P                   # 4 token tiles
ND = D // P                   # 4 contraction chunks over D
NS = S // P                   # 8 slot chunks
NEG = -1e30
QD_TILES = [(0, 128), (128, 128), (256, 64)]

f32 = mybir.dt.float32
f32r = mybir.dt.float32r
bf16 = mybir.dt.bfloat16
AF = mybir.ActivationFunctionType
OP = mybir.AluOpType

_CACHE: dict = {}


def _build_nc():
    nc = bacc.Bacc("TRN2", target_bir_lowering=False, debug=False)

    dr = {}

    def din(name, shape, dt_):
        dr[name] = nc.dram_tensor(name, shape, dt_, kind="ExternalInput")

    din("queryT", (QD, T), f32)
    din("WqpT", (QD, D), f32)
    din("WqT", (D, D), f32r)
    din("WkT", (D, D), f32r)
    din("WvT", (D, D), f32r)
    din("WoT", (D, D), f32r)
    din("WoutT", (D, QD), f32r)
    din("memkT", (D, S), f32)
    din("memkTr", (D, S), f32r)
    din("memvT", (D, S), f32r)
    din("ln_g", (D,), f32)
    din("ln_b", (D,), f32)
    din("bout", (384,), f32)
    out_dram = nc.dram_tensor("outT", (QD, T), f32, kind="ExternalOutput")

    with tile.TileContext(nc) as tc:
        with (
            tc.tile_pool(name="const", bufs=1) as const,
            tc.tile_pool(name="main", bufs=1) as main,
            tc.tile_pool(name="scr2", bufs=2) as scr2,
            tc.tile_pool(name="wpool", bufs=4) as wpool,
            tc.tile_pool(name="psmm", bufs=2, space="PSUM") as psmm,
            tc.tile_pool(name="psq", bufs=2, space="PSUM") as psq,
            tc.tile_pool(name="psctx", bufs=1, space="PSUM") as psctx,
            tc.tile_pool(name="psaux", bufs=1, space="PSUM") as psaux,
            nc.allow_low_precision(reason="validated f32r/bf16 paths"),
        ):
            # ---------- constants ----------
            ident = const.tile([P, P], bf16, tag="ident")
            make_identity(nc, ident)
            ones_col = const.tile([P, 1], f32, tag="ones_col")
            nc.vector.memset(ones_col, 1.0)
            ones_row = const.tile([1, P], f32, tag="ones_row")
            nc.vector.memset(ones_row, 1.0)
            # selA/selB rows for per-head-pair denominator broadcast
            halfsel = const.tile([1, 2 * P], f32, tag="halfsel")
            nc.vector.memset(halfsel, 0.0)
            nc.vector.memset(halfsel[0:1, 64:192], 1.0)
            halfsel_r = const.tile([1, 2 * P], f32r, tag="halfsel_r")
            nc.scalar.copy(halfsel_r[:], halfsel[:])
            # layout: [0:64]=0, [64:192]=1, [192:256]=0
            ones_row_r = halfsel_r[0:1, 64:192]  # [1,128] ones
            selA = halfsel_r[0:1, 128:256]       # ones x64, zeros x64
            selB = halfsel_r[0:1, 0:128]         # zeros x64, ones x64
            ones_col_r = const.tile([P, 1], f32r, tag="ones_col_r")
            nc.scalar.copy(ones_col_r[:], ones_col[:])
            eps_tab = const.tile([P, 1], f32, tag="eps_tab")
            nc.vector.memset(eps_tab, 1e-12)
            eps_ln = const.tile([1, 1], f32, tag="eps_ln")
            nc.vector.memset(eps_ln, EPS)

            # PE p-state warmup: dead transposes with no DMA dependency keep
            # the tensor engine busy from ~1.7us so the ramp-to-full-clock
            # window burns off before the real fp32 matmuls arrive.
            ps_warm = psaux.tile([P, P], bf16, tag="aux", name="warm")
            for _ in range(22):
                nc.tensor.matmul(
                    ps_warm, lhsT=ident, rhs=ident,
                    is_transpose=True, skip_group_check=True,
                )

            # ---------- DMA loads (critical tensors first) ----------
            def load_qd_tiles(name, cols, tags, dt_):
                tiles = []
                for (off, sz), tag in zip(QD_TILES, tags):
                    t_ = main.tile([sz, cols], dt_, tag=tag, name=f"ld_{tag}")
                    nc.sync.dma_start(t_[:], dr[name].ap()[ds(off, sz), :])
                    tiles.append(t_)
                return tiles

            def load_wide(name, inner, dt_, tag):
                # (a p) s -> p (a s): one descriptor for a (ND*P, inner) tensor
                t_ = main.tile([P, ND, inner], dt_, tag=tag, name=f"ld_{tag}")
                nc.sync.dma_start(
                    t_[:], dr[name].ap().rearrange("(a p) s -> p a s", p=P)
                )
                return t_

            qryT = load_qd_tiles("queryT", T, ["qry0", "qry1", "qry2"], f32)
            wqpT = load_qd_tiles("WqpT", D, ["wqp0", "wqp1", "wqp2"], f32)
            ktab = load_wide("memkT", S, f32, "ktab")       # [128, dc, S]
            wq = load_wide("WqT", D, f32r, "wq")            # [128, dc, D]
            wk = load_wide("WkT", D, f32r, "wk")
            vtab = load_wide("memvT", S, f32r, "vtab")
            ktabr = load_wide("memkTr", S, f32r, "ktabr")
            wv = load_wide("WvT", D, f32r, "wv")
            wo = load_wide("WoT", D, f32r, "wo")
            wout = load_wide("WoutT", QD, f32r, "wout")     # [128, dc, QD]

            g_sb = const.tile([P, ND], f32, tag="g")
            nc.sync.dma_start(g_sb[:], dr["ln_g"].ap().rearrange("(o p) -> p o", p=P))
            b_sb = const.tile([P, ND], f32, tag="b")
            nc.sync.dma_start(b_sb[:], dr["ln_b"].ap().rearrange("(o p) -> p o", p=P))
            bout_sb = const.tile([P, 3], f32, tag="bout")
            nc.sync.dma_start(bout_sb[:], dr["bout"].ap().rearrange("(o p) -> p o", p=P))

            # ---------- key inverse-norms (replicated), off the PE ----------
            # rsqB[p, s] = 1/sqrt(sum_d k[d,s]^2 + 1e-12), same value on all
            # partitions p (partition_all_reduce replicates its output).
            def inv_norms(tab, acc_tag, tag_out, use_lnexp=False):
                # square+sum in [P, T]-column halves to keep scratch small
                acc = main.tile([P, S], f32, tag=acc_tag, name=f"{tag_out}_acc")
                for half in range(2):
                    col = ds(half * T, T)
                    for i in range(ND):
                        src_ap = tab[:, i, col]
                        if src_ap.dtype != f32:
                            src_ap = src_ap.bitcast(f32)
                        if i == 0:
                            nc.scalar.square(acc[:, col], src_ap)
                        else:
                            sq0 = scr2.tile([P, T], f32, tag="sq", name=f"{tag_out}_sq")
                            nc.scalar.square(sq0[:], src_ap)
                            nc.gpsimd.tensor_tensor(acc[:, col], acc[:, col], sq0[:], OP.add)
                red = main.tile([P, S], f32, tag=tag_out, name=tag_out)
                nc.gpsimd.partition_all_reduce(
                    red[:], acc[:], channels=P, reduce_op=bass_isa.ReduceOp.add
                )
                if use_lnexp:
                    # rsqrt via exp(-0.5 ln(x+eps)): stays in the ln/exp ACT
                    # function set (no table switch); fine off the selection path
                    nc.scalar.activation(red[:], red[:], AF.Ln, bias=eps_tab[:])
                    nc.scalar.activation(red[:], red[:], AF.Exp, scale=-0.5)
                else:
                    # exact-class: sqrt on ACT, Newton reciprocal on DVE
                    nc.scalar.activation(red[:], red[:], AF.Sqrt, bias=eps_tab[:])
                    nc.vector.reciprocal(red[:], red[:])
                return red

            rsqB = inv_norms(ktab, "kp0", "rsqB")

            # ---------- qT[d, t] = Wqp @ query.T (exact fp32) ----------
            qT = []
            for dt_i in range(ND):
                t_ = main.tile([P, T], f32, tag=f"qt{dt_i}", name=f"q{dt_i}")
                ps = psmm.tile([P, T], f32, tag="mm")
                for c in range(3):
                    nc.tensor.matmul(
                        ps, lhsT=wqpT[c][:, ts(dt_i, P)], rhs=qryT[c][:],
                        start=(c == 0), stop=(c == 2),
                    )
                nc.scalar.copy(t_[:], ps)
                qT.append(t_)
            qTr = []
            for dt_i in range(ND):
                tr_ = main.tile([P, T], f32r, tag=f"qtr{dt_i}", name=f"qr{dt_i}")
                nc.vector.tensor_copy(tr_[:], qT[dt_i][:])
                qTr.append(tr_)


            # ---------- scores[t, s] = (q @ k_raw.T) * rsq  (exact fp32) ------
            sc = []
            for tt in range(NT):
                t_ = main.tile([P, S], f32, tag=f"sc{tt}", name=f"sc{tt}")
                for half in range(2):
                    ps = psmm.tile([P, T], f32, tag="mm")
                    for dc in range(ND):
                        nc.tensor.matmul(
                            ps,
                            lhsT=qT[dc][:, ts(tt, P)],
                            rhs=ktab[:, dc, ds(half * T, T)],
                            start=(dc == 0), stop=(dc == ND - 1),
                        )
                    nc.scalar.copy(t_[:, ds(half * T, T)], ps)
                    # normalize on Pool in SBUF (GPSIMD cannot read PSUM)
                    nc.gpsimd.tensor_tensor(
                        t_[:, ds(half * T, T)], t_[:, ds(half * T, T)],
                        rsqB[:, ds(half * T, T)], OP.mult,
                    )
                sc.append(t_)

                # top-32 threshold per token row (4 rounds of max8), 0/1 mask
                work = main.tile([P, S], f32, tag=f"wk{tt % 2}", name=f"wk{tt}")
                cur = t_
                for r in range(4):
                    mx = main.tile([P, 8], f32, tag=f"mx{tt}_{r}", name=f"mx{tt}_{r}")
                    nc.vector.max(out=mx[:], in_=cur[:])
                    if r < 3:
                        nc.vector.match_replace(
                            out=work[:], in_to_replace=mx[:], in_values=cur[:],
                            imm_value=NEG,
                        )
                        cur = work
                mk_tags = ["qry0", "qry1", "wqp0", "wqp1"]
                m_ = main.tile([P, S], bf16, tag=mk_tags[tt], name=f"mk{tt}")
                nc.vector.tensor_scalar(
                    m_[:], t_[:], mx[:, 7:8], None, op0=OP.is_ge
                )
                sc.append(m_)
            mask01 = [sc[2 * tt + 1] for tt in range(NT)]

            # ---------- value inverse-norms (Pool, off critical path) ----
            rsvB = inv_norms(vtab, "kp1", "rsvB", use_lnexp=True)

            # ---------- KpT[e, s] = Wk @ k_raw.T (f32r), scaled in-place -----
            kp = []
            for e in range(ND):
                t_ = main.tile([P, S], f32r, tag=f"kp{e}", name=f"kp{e}")
                for half in range(2):
                    ps = psmm.tile([P, T], f32, tag="mm")
                    for dc in range(ND):
                        nc.tensor.matmul(
                            ps,
                            lhsT=wk[:, dc, ts(e, P)],
                            rhs=ktabr[:, dc, ds(half * T, T)],
                            start=(dc == 0), stop=(dc == ND - 1),
                        )
                    nc.scalar.copy(t_[:, ds(half * T, T)], ps)
                # column scale by 1/|k_s| in place (Pool)
                nc.gpsimd.tensor_tensor(t_[:], t_[:].bitcast(f32), rsqB[:], OP.mult)
                kp.append(t_)

            # rsv in [slot-partition, 1] layout per chunk: 8 tiny PE transposes
            ps_rsv = psaux.tile([P, 8], f32, tag="aux", name="ps_rsv")
            for st in range(NS):
                nc.tensor.matmul(
                    ps_rsv[:, st : st + 1],
                    lhsT=rsvB[0:1, ts(st, P)], rhs=ones_row[0:1, 0:1],
                    is_transpose=True, skip_group_check=True,
                )
            rsv_sb = const.tile([P, 8], f32, tag="rsv_sb")
            nc.vector.tensor_copy(rsv_sb[:], ps_rsv)

            # ---------- Vp[s, 8 heads x (64 + ones)] = valsn @ Wv.T (bf16) ----
            vp = []
            for st in range(NS):
                t_ = main.tile([P, H, DH + 1], bf16, tag=f"vp{st}", name=f"vp{st}")
                nc.vector.memset(t_[:, :, DH : DH + 1], 1.0)
                ps = psmm.tile([P, D], f32, tag="mm")
                for dc in range(ND):
                    nc.tensor.matmul(
                        ps,
                        lhsT=vtab[:, dc, ts(st, P)],
                        rhs=wv[:, dc, :],
                        start=(dc == 0), stop=(dc == ND - 1),
                    )
                nc.scalar.activation(
                    t_[:, :, 0:DH], ps.rearrange("p (h e) -> p h e", h=H),
                    AF.Copy, scale=rsv_sb[:, st : st + 1],
                )
                vp.append(t_)

            # ---------- qhT[e, t] = (Wq @ qT) / 8  (f32r) ----------
            qh = []
            for e in range(ND):
                t_ = main.tile([P, T], f32r, tag=f"qh{e}", name=f"qh{e}")
                ps = psmm.tile([P, T], f32, tag="mm")
                for dc in range(ND):
                    nc.tensor.matmul(
                        ps, lhsT=wq[:, dc, ts(e, P)], rhs=qTr[dc][:],
                        start=(dc == 0), stop=(dc == ND - 1),
                    )
                nc.scalar.mul(t_[:], ps, 1.0 / np.sqrt(DH))
                qh.append(t_)

            # ---------- transpose masks to [s, t] (bf16 PE transposes) -------
            # mTq[g][:, i, :] = mask chunk (4g+i) transposed, full T columns
            mTq = []
            for g in range(2):
                mt = main.tile([P, 4, T], bf16, tag=f"sc{g}", name=f"mTq{g}")
                for i in range(4):
                    ps_t = psaux.tile([P, T], bf16, tag="aux", name=f"pst{g}{i}")
                    for tt in range(NT):
                        nc.tensor.matmul(
                            ps_t[:, ts(tt, P)],
                            lhsT=mask01[tt][:, ts(4 * g + i, P)],
                            rhs=ident, is_transpose=True, skip_group_check=True,
                        )
                    nc.vector.tensor_copy(mt[:, i, :], ps_t)
                mTq.append(mt)

            # ---------- attention: per 256-token half, quads of 4 chunks -----
            # ctxT[et][ro:ro+DH, t] per head; epilogue per half underneath the
            # other half's attention.
            cx_tags = ["sc2", "sc3", "wk0", "wk1"]
            ctxT = [
                main.tile([P, T], f32, tag=cx_tags[dt_i], name=f"cx{dt_i}")
                for dt_i in range(ND)
            ]
            oT_big = main.tile([P, ND, T], f32, tag="vtab", name="oT")
            oT = [oT_big[:, dt_i, :] for dt_i in range(ND)]

            def attention_half(half, interleave=None):
                tok = ds(half * HT, HT)
                for h in range(H):
                    et, ro = h // 2, (h % 2) * 64
                    if h % 2 == 0:
                        den_pair = scr2.tile([1, 2 * HT], f32r, tag="den")
                    ps_ctx = psctx.tile([DH + 1, HT], f32, tag="ctx")
                    for g in range(2):
                        ps_att = psq.tile([P, 4, HT], f32, tag="q")
                        for i in range(4):
                            nc.tensor.matmul(
                                ps_att[:, i, :],
                                lhsT=kp[et][ro : ro + DH, ts(4 * g + i, P)],
                                rhs=qh[et][ro : ro + DH, tok],
                                start=True, stop=True, skip_group_check=True,
                            )
                        w = scr2.tile([P, 4, HT], bf16, tag="u")
                        nc.scalar.activation(w[:], ps_att, AF.Exp)
                        nc.vector.tensor_tensor(
                            w[:], w[:], mTq[g][:, :, tok], OP.mult
                        )
                        for i in range(4):
                            nc.tensor.matmul(
                                ps_ctx, lhsT=vp[4 * g + i][:, h, :], rhs=w[:, i, :],
                                start=(g == 0 and i == 0), stop=(g == 1 and i == 3),
                            )
                    nc.vector.tensor_copy(
                        ctxT[et][ro : ro + DH, tok].bitcast(f32r), ps_ctx[0:DH, :]
                    )
                    nc.vector.reciprocal(
                        den_pair[0:1, ds((h % 2) * HT, HT)], ps_ctx[DH : DH + 1, :]
                    )
                    if h % 2 == 1:
                        ps_rb = psaux.tile([P, HT], f32, tag="aux")
                        nc.tensor.matmul(
                            ps_rb, lhsT=selA, rhs=den_pair[0:1, 0:HT],
                            start=True, stop=False,
                        )
                        nc.tensor.matmul(
                            ps_rb, lhsT=selB, rhs=den_pair[0:1, HT : 2 * HT],
                            start=False, stop=True,
                        )
                        nc.vector.tensor_tensor(
                            ctxT[et][:, tok].bitcast(f32r), ctxT[et][:, tok],
                            ps_rb, OP.mult,
                        )
                        if interleave is not None:
                            interleave[h // 2]()

            def epilogue_parts(half):
                parts = []
                tok = ds(half * HT, HT)
                st = {}

                def part_a():
                    # oT[e, t] = Wo @ ctx.T (f32r)
                    for e in range(ND):
                        ps = psmm.tile([P, T], f32, tag="mm")
                        for dc in range(ND):
                            nc.tensor.matmul(
                                ps[:, 0:HT], lhsT=wo[:, dc, ts(e, P)],
                                rhs=ctxT[dc][:, tok].bitcast(f32r),
                                start=(dc == 0), stop=(dc == ND - 1),
                            )
                        nc.scalar.copy(oT[e][:, tok].bitcast(f32r), ps[:, 0:HT])

                def part_b():
                    # LayerNorm stats via f32r ones-matmuls
                    ps_mu = psaux.tile([P, T], f32, tag="aux")
                    st["ps_mu"] = ps_mu
                    sqs = []
                    for dc in range(ND):
                        sq = scr2.tile([P, HT], f32r, tag="lnsq")
                        nc.scalar.square(sq, oT[dc][:, tok])
                        sqs.append(sq)
                    for dc in range(ND):
                        nc.tensor.matmul(
                            ps_mu[0:1, 0:HT], lhsT=ones_col_r[:],
                            rhs=oT[dc][:, tok].bitcast(f32r),
                            start=(dc == 0), stop=(dc == ND - 1),
                            skip_group_check=True,
                        )
                    for dc in range(ND):
                        nc.tensor.matmul(
                            ps_mu[0:1, HT : 2 * HT], lhsT=ones_col_r[:],
                            rhs=sqs[dc][:],
                            start=(dc == 0), stop=(dc == ND - 1),
                            skip_group_check=True,
                        )

                def part_c():
                    ps_mu = st["ps_mu"]
                    mu_row = main.tile([1, HT], f32, tag="mu", name=f"mu{half}")
                    ms_row = main.tile([1, HT], f32, tag="ms", name=f"ms{half}")
                    nc.scalar.mul(mu_row[:], ps_mu[0:1, 0:HT], 1.0 / D)
                    nc.scalar.mul(ms_row[:], ps_mu[0:1, HT : 2 * HT], 1.0 / D)
                    var_row = main.tile([1, HT], f32, tag="var", name=f"var{half}")
                    nc.vector.tensor_tensor(var_row[:], mu_row[:], mu_row[:], OP.mult)
                    nc.vector.tensor_sub(var_row[:], ms_row[:], var_row[:])
                    rstd_row = main.tile([1, HT], f32r, tag="rstd", name=f"rstd{half}")
                    nc.scalar.activation(var_row[:], var_row[:], AF.Ln, bias=eps_ln[:])
                    nc.scalar.activation(rstd_row[:], var_row[:], AF.Exp, scale=-0.5)
                    crow = main.tile([1, HT], f32r, tag="crow", name=f"crow{half}")
                    nc.vector.scalar_tensor_tensor(
                        crow[:], mu_row[:], -1.0, rstd_row[:].bitcast(f32),
                        op0=OP.mult, op1=OP.mult
                    )
                    ps_b = psaux.tile([P, T], f32, tag="aux")
                    nc.tensor.matmul(
                        ps_b[:, 0:HT], lhsT=ones_row_r, rhs=rstd_row[:],
                        start=True, stop=True, skip_group_check=True,
                    )
                    nc.tensor.matmul(
                        ps_b[:, HT : 2 * HT], lhsT=ones_row_r, rhs=crow[:],
                        start=True, stop=True, skip_group_check=True,
                    )
                    nrm = []
                    for dt_i in range(ND):
                        nc.vector.tensor_tensor(
                            oT[dt_i][:, tok].bitcast(f32r), oT[dt_i][:, tok],
                            ps_b[:, 0:HT], OP.mult,
                        )
                        nc.vector.tensor_tensor(
                            oT[dt_i][:, tok].bitcast(f32r), oT[dt_i][:, tok],
                            ps_b[:, HT : 2 * HT], OP.add,
                        )
                        n_ = scr2.tile([P, HT], f32r, tag=f"nrm{dt_i % 2}")
                        nc.vector.scalar_tensor_tensor(
                            n_[:], oT[dt_i][:, tok], g_sb[:, dt_i : dt_i + 1],
                            b_sb[:, dt_i : dt_i + 1].to_broadcast([P, HT]),
                            op0=OP.mult, op1=OP.add,
                        )
                        nrm.append(n_)
                    st["nrm"] = nrm

                def part_d():
                    nrm = st["nrm"]
                    for qt, (off, sz) in enumerate(QD_TILES):
                        ps = psmm.tile([P, T], f32, tag="mm")
                        for e in range(ND):
                            nc.tensor.matmul(
                                ps[:sz, 0:HT], lhsT=wout[:, e, ds(off, sz)],
                                rhs=nrm[e][:],
                                start=(e == 0), stop=(e == ND - 1),
                            )
                        ot_sb = scr2.tile([P, HT], f32, tag="ot")
                        nc.scalar.add(
                            ot_sb[:sz, :], ps[:sz, 0:HT], bout_sb[:sz, qt : qt + 1]
                        )
                        nc.sync.dma_start(
                            out_dram.ap()[ds(off, sz), ds(half * HT, HT)],
                            ot_sb[:sz, :],
                        )

                return [part_a, part_b, part_c, part_d]

            attention_half(0)
            attention_half(1, interleave=epilogue_parts(0))
            for p_ in epilogue_parts(1):
                p_()

    nc.compile()
    return nc


def _prep_in_maps(inputs):
    def c(a):
        return np.ascontiguousarray(a, dtype=np.float32)

    q = np.asarray(inputs["query_states"], dtype=np.float32).reshape(B * N, QD)
    shared = {
        "WqpT": c(np.asarray(inputs["Wqp"]).T),
        "WqT": c(np.asarray(inputs["Wq"]).T),
        "WkT": c(np.asarray(inputs["Wk"]).T),
        "WvT": c(np.asarray(inputs["Wv"]).T),
        "WoT": c(np.asarray(inputs["Wo"]).T),
        "WoutT": c(np.asarray(inputs["Wout"]).T),
        "memkT": c(np.asarray(inputs["mem_keys"]).T),
        "memkTr": c(np.asarray(inputs["mem_keys"]).T),
        "memvT": c(np.asarray(inputs["mem_values"]).T),
        "ln_g": c(np.asarray(inputs["ln_g"])),
        "ln_b": c(np.asarray(inputs["ln_b"])),
        "bout": c(np.pad(np.asarray(inputs["bout"]), (0, 384 - QD))),
    }
    in_maps = []
    for core in range(NCORES):
        m = dict(shared)
        m["queryT"] = c(q[core * T : (core + 1) * T, :].T)
        in_maps.append(m)
    return in_maps


def kernel(**inputs) -> np.ndarray:
    if "nc" not in _CACHE:
        _CACHE["nc"] = _build_nc()
    nc = _CACHE["nc"]
    in_maps = _prep_in_maps(inputs)
    res = run_bass_kernel_spmd(nc, in_maps, core_ids=list(range(NCORES)))
    out = np.empty((B * N, QD), dtype=np.float32)
    for core in range(NCORES):
        out[core * T : (core + 1) * T, :] = res.results[core]["outT"].T
    return out.reshape(B, N, QD)


# revision 18
# speedup vs baseline: 1.1718x; 1.0242x over previous
"""GatedLTMMemory kernel for 8 Trainium2 NeuronCores.

Data-parallel over the 4096 flattened (B,N) tokens: 512 tokens per core.
Memory-slot tables and weights are replicated. Per-selected-slot projections
are replaced by projecting the slot tables once and running a masked
full-softmax over all S slots (exactly equivalent math).

v2 restructure vs the first working kernel (183.9us):
  - slot-table l2 normalization is OFF the critical path: raw scores
    q.k are computed straight after the DMAs land; the 1/|k| column scale
    is applied on the PSUM->SBUF copy (DVE). Norms come from ACT squares +
    Pool adds + GPSIMD partition_all_reduce (output replicated across
    partitions, so no PE broadcast matmul is needed).
  - single-descriptor DMA loads ((a p) d -> p (a d) rearrange) cut HWDGE
    descriptor serialization from ~25us to ~9us.
  - attention is split into two 256-token halves; within a half, 4 slot
    chunks share one [128,1024] PSUM quad -> one 1024-wide exp. The
    epilogue (Wo+LN+Wout) of half 0 runs under the attention of half 1.
  - LayerNorm stats, Wo/Wout, attention all f32r; fp32 only where the
    top-32 selection needs it (qT, scores, key norms).
  - f32r operands are bitcast views of fp32 tiles (no convert copies).
"""

import numpy as np

import concourse.bacc as bacc
import concourse.bass_isa as bass_isa
import concourse.mybir as mybir
import concourse.tile as tile
from concourse.bass import ds, ts
from concourse.bass_utils import run_bass_kernel_spmd
from concourse.masks import make_identity

B, N, QD, D, S, H, K = 4, 1024, 320, 512, 1024, 8, 32
DH = D // H
EPS = 1e-5
P = 128
T = 512                       # tokens per core
HT = 256                      # tokens per epilogue half
NCORES = 8
NT = T // P                   # 4 token tiles
ND = D // P                   # 4 contraction chunks over D
NS = S // P                   # 8 slot chunks
NEG = -1e30
QD_TILES = [(0, 128), (128, 128), (256, 64)]

f32 = mybir.dt.float32
f32r = mybir.dt.float32r
bf16 = mybir.dt.bfloat16
AF = mybir.ActivationFunctionType
OP = mybir.AluOpType

_CACHE: dict = {}


def _build_nc():
    nc = bacc.Bacc("TRN2", target_bir_lowering=False, debug=False)

    dr = {}

    def din(name, shape, dt_):
        dr[name] = nc.dram_tensor(name, shape, dt_, kind="ExternalInput")

    din("queryT", (QD, T), f32)
    din("WqpT", (QD, D), f32)
    din("WqT", (D, D), f32r)
    din("WkT", (D, D), f32r)
    din("WvT", (D, D), f32r)
    din("WoT", (D, D), f32r)
    din("WoutT", (D, QD), f32r)
    din("memkT", (D, S), f32)
    din("memkTr", (D, S), f32r)
    din("memvT", (D, S), f32r)
    din("ln_g", (D,), f32)
    din("ln_b", (D,), f32)
    din("bout", (384,), f32)
    out_dram = nc.dram_tensor("outT", (QD, T), f32, kind="ExternalOutput")

    with tile.TileContext(nc) as tc:
        with (
            tc.tile_pool(name="const", bufs=1) as const,
            tc.tile_pool(name="main", bufs=1) as main,
            tc.tile_pool(name="scr2", bufs=2) as scr2,
            tc.tile_pool(name="wpool", bufs=4) as wpool,
            tc.tile_pool(name="psmm", bufs=2, space="PSUM") as psmm,
            tc.tile_pool(name="psq", bufs=2, space="PSUM") as psq,
            tc.tile_pool(name="psctx", bufs=1, space="PSUM") as psctx,
            tc.tile_pool(name="psaux", bufs=1, space="PSUM") as psaux,
            nc.allow_low_precision(reason="validated f32r/bf16 paths"),
        ):
            # ---------- constants ----------
            ident = const.tile([P, P], bf16, tag="ident")
            make_identity(nc, ident)
            ones_col = const.tile([P, 1], f32, tag="ones_col")
            nc.vector.memset(ones_col, 1.0)
            ones_row = const.tile([1, P], f32, tag="ones_row")
            nc.vector.memset(ones_row, 1.0)
            # selA/selB rows for per-head-pair denominator broadcast
            halfsel = const.tile([1, 2 * P], f32, tag="halfsel")
            nc.vector.memset(halfsel, 0.0)
            nc.vector.memset(halfsel[0:1, 64:192], 1.0)
            halfsel_r = const.tile([1, 2 * P], f32r, tag="halfsel_r")
            nc.scalar.copy(halfsel_r[:], halfsel[:])
            # layout: [0:64]=0, [64:192]=1, [192:256]=0
            ones_row_r = halfsel_r[0:1, 64:192]  # [1,128] ones
            selA = halfsel_r[0:1, 128:256]       # ones x64, zeros x64
            selB = halfsel_r[0:1, 0:128]         # zeros x64, ones x64
            ones_col_r = const.tile([P, 1], f32r, tag="ones_col_r")
            nc.scalar.copy(ones_col_r[:], ones_col[:])
            eps_tab = const.tile([P, 1], f32, tag="eps_tab")
            nc.vector.memset(eps_tab, 1e-12)
            eps_ln = const.tile([1, 1], f32, tag="eps_ln")
            nc.vector.memset(eps_ln, EPS)

            # PE p-state warmup: dead transposes with no DMA dependency keep
            # the tensor engine busy from ~1.7us so the ramp-to-full-clock
            # window burns off before the real fp32 matmuls arrive.
            ps_warm = psaux.tile([P, P], bf16, tag="aux", name="warm")
            for _ in range(22):
                nc.tensor.matmul(
                    ps_warm, lhsT=ident, rhs=ident,
                    is_transpose=True, skip_group_check=True,
                )

            # ---------- DMA loads (critical tensors first) ----------
            def load_qd_tiles(name, cols, tags, dt_):
                tiles = []
                for (off, sz), tag in zip(QD_TILES, tags):
                    t_ = main.tile([sz, cols], dt_, tag=tag, name=f"ld_{tag}")
                    nc.sync.dma_start(t_[:], dr[name].ap()[ds(off, sz), :])
                    tiles.append(t_)
                return tiles

            def load_wide(name, inner, dt_, tag):
                # (a p) s -> p (a s): one descriptor for a (ND*P, inner) tensor
                t_ = main.tile([P, ND, inner], dt_, tag=tag, name=f"ld_{tag}")
                nc.sync.dma_start(
                    t_[:], dr[name].ap().rearrange("(a p) s -> p a s", p=P)
                )
                return t_

            qryT = load_qd_tiles("queryT", T, ["qry0", "qry1", "qry2"], f32)
            wqpT = load_qd_tiles("WqpT", D, ["wqp0", "wqp1", "wqp2"], f32)
            ktab = load_wide("memkT", S, f32, "ktab")       # [128, dc, S]
            wq = load_wide("WqT", D, f32r, "wq")            # [128, dc, D]
            wk = load_wide("WkT", D, f32r, "wk")
            vtab = load_wide("memvT", S, f32r, "vtab")
            ktabr = load_wide("memkTr", S, f32r, "ktabr")
            wv = load_wide("WvT", D, f32r, "wv")
            wo = load_wide("WoT", D, f32r, "wo")
            wout = load_wide("WoutT", QD, f32r, "wout")     # [128, dc, QD]

            g_sb = const.tile([P, ND], f32, tag="g")
            nc.sync.dma_start(g_sb[:], dr["ln_g"].ap().rearrange("(o p) -> p o", p=P))
            b_sb = const.tile([P, ND], f32, tag="b")
            nc.sync.dma_start(b_sb[:], dr["ln_b"].ap().rearrange("(o p) -> p o", p=P))
            bout_sb = const.tile([P, 3], f32, tag="bout")
            nc.sync.dma_start(bout_sb[:], dr["bout"].ap().rearrange("(o p) -> p o", p=P))

            # ---------- key inverse-norms (replicated), off the PE ----------
            # rsqB[p, s] = 1/sqrt(sum_d k[d,s]^2 + 1e-12), same value on all
            # partitions p (partition_all_reduce replicates its output).
            def inv_norms(tab, acc_tag, tag_out, use_lnexp=False):
                # square+sum in [P, T]-column halves to keep scratch small
                acc = main.tile([P, S], f32, tag=acc_tag, name=f"{tag_out}_acc")
                for half in range(2):
                    col = ds(half * T, T)
                    for i in range(ND):
                        src_ap = tab[:, i, col]
                        if src_ap.dtype != f32:
                            src_ap = src_ap.bitcast(f32)
                        if i == 0:
                            nc.scalar.square(acc[:, col], src_ap)
                        else:
                            sq0 = scr2.tile([P, T], f32, tag="sq", name=f"{tag_out}_sq")
                            nc.scalar.square(sq0[:], src_ap)
                            nc.gpsimd.tensor_tensor(acc[:, col], acc[:, col], sq0[:], OP.add)
                red = main.tile([P, S], f32, tag=tag_out, name=tag_out)
                nc.gpsimd.partition_all_reduce(
                    red[:], acc[:], channels=P, reduce_op=bass_isa.ReduceOp.add
                )
                if use_lnexp:
                    # rsqrt via exp(-0.5 ln(x+eps)): stays in the ln/exp ACT
                    # function set (no table switch); fine off the selection path
                    nc.scalar.activation(red[:], red[:], AF.Ln, bias=eps_tab[:])
                    nc.scalar.activation(red[:], red[:], AF.Exp, scale=-0.5)
                else:
                    # exact-class: sqrt on ACT, Newton reciprocal on DVE
                    nc.scalar.activation(red[:], red[:], AF.Sqrt, bias=eps_tab[:])
                    nc.vector.reciprocal(red[:], red[:])
                return red

            rsqB = inv_norms(ktab, "kp0", "rsqB")

            # ---------- qT[d, t] = Wqp @ query.T (exact fp32) ----------
            qT = []
            for dt_i in range(ND):
                t_ = main.tile([P, T], f32, tag=f"qt{dt_i}", name=f"q{dt_i}")
                ps = psmm.tile([P, T], f32, tag="mm")
                for c in range(3):
                    nc.tensor.matmul(
                        ps, lhsT=wqpT[c][:, ts(dt_i, P)], rhs=qryT[c][:],
                        start=(c == 0), stop=(c == 2),
                    )
                nc.scalar.copy(t_[:], ps)
                qT.append(t_)
            qTr = []
            for dt_i in range(ND):
                tr_ = main.tile([P, T], f32r, tag=f"qtr{dt_i}", name=f"qr{dt_i}")
                nc.vector.tensor_copy(tr_[:], qT[dt_i][:])
                qTr.append(tr_)


            # ---------- scores[t, s] = (q @ k_raw.T) * rsq  (exact fp32) ------
            sc = []
            for tt in range(NT):
                t_ = main.tile([P, S], f32, tag=f"sc{tt}", name=f"sc{tt}")
                for half in range(2):
                    ps = psmm.tile([P, T], f32, tag="mm")
                    for dc in range(ND):
                        nc.tensor.matmul(
                            ps,
                            lhsT=qT[dc][:, ts(tt, P)],
                            rhs=ktab[:, dc, ds(half * T, T)],
                            start=(dc == 0), stop=(dc == ND - 1),
                        )
                    nc.scalar.copy(t_[:, ds(half * T, T)], ps)
                    # normalize on Pool in SBUF (GPSIMD cannot read PSUM)
                    nc.gpsimd.tensor_tensor(
                        t_[:, ds(half * T, T)], t_[:, ds(half * T, T)],
                        rsqB[:, ds(half * T, T)], OP.mult,
                    )
                sc.append(t_)

                # top-32 threshold per token row (4 rounds of max8), 0/1 mask
                work = main.tile([P, S], f32, tag=f"wk{tt % 2}", name=f"wk{tt}")
                cur = t_
                for r in range(4):
                    mx = main.tile([P, 8], f32, tag=f"mx{tt}_{r}", name=f"mx{tt}_{r}")
                    nc.vector.max(out=mx[:], in_=cur[:])
                    if r < 3:
                        nc.vector.match_replace(
                            out=work[:], in_to_replace=mx[:], in_values=cur[:],
                            imm_value=NEG,
                        )
                        cur = work
                mk_tags = ["qry0", "qry1", "wqp0", "wqp1"]
                m_ = main.tile([P, S], bf16, tag=mk_tags[tt], name=f"mk{tt}")
                nc.vector.tensor_scalar(
                    m_[:], t_[:], mx[:, 7:8], None, op0=OP.is_ge
                )
                sc.append(m_)
            mask01 = [sc[2 * tt + 1] for tt in range(NT)]

            # ---------- value inverse-norms (Pool, off critical path) ----
            rsvB = inv_norms(vtab, "kp1", "rsvB")

            # ---------- KpT[e, s] = Wk @ k_raw.T (f32r), scaled in-place -----
            kp = []
            for e in range(ND):
                t_ = main.tile([P, S], f32r, tag=f"kp{e}", name=f"kp{e}")
                for half in range(2):
                    ps = psmm.tile([P, T], f32, tag="mm")
                    for dc in range(ND):
                        nc.tensor.matmul(
                            ps,
                            lhsT=wk[:, dc, ts(e, P)],
                            rhs=ktabr[:, dc, ds(half * T, T)],
                            start=(dc == 0), stop=(dc == ND - 1),
                        )
                    nc.scalar.copy(t_[:, ds(half * T, T)], ps)
                # column scale by 1/|k_s| in place (Pool)
                nc.gpsimd.tensor_tensor(t_[:], t_[:].bitcast(f32), rsqB[:], OP.mult)
                kp.append(t_)

            # rsv in [slot-partition, 1] layout per chunk: 8 tiny PE transposes
            ps_rsv = psaux.tile([P, 8], f32, tag="aux", name="ps_rsv")
            for st in range(NS):
                nc.tensor.matmul(
                    ps_rsv[:, st : st + 1],
                    lhsT=rsvB[0:1, ts(st, P)], rhs=ones_row[0:1, 0:1],
                    is_transpose=True, skip_group_check=True,
                )
            rsv_sb = const.tile([P, 8], f32, tag="rsv_sb")
            nc.vector.tensor_copy(rsv_sb[:], ps_rsv)

            # ---------- Vp[s, 8 heads x (64 + ones)] = valsn @ Wv.T (bf16) ----
            vp = []
            for st in range(NS):
                t_ = main.tile([P, H, DH + 1], bf16, tag=f"vp{st}", name=f"vp{st}")
                nc.vector.memset(t_[:, :, DH : DH + 1], 1.0)
                ps = psmm.tile([P, D], f32, tag="mm")
                for dc in range(ND):
                    nc.tensor.matmul(
                        ps,
                        lhsT=vtab[:, dc, ts(st, P)],
                        rhs=wv[:, dc, :],
                        start=(dc == 0), stop=(dc == ND - 1),
                    )
                nc.vector.tensor_scalar(
                    t_[:, :, 0:DH], ps.rearrange("p (h e) -> p h e", h=H),
                    rsv_sb[:, st : st + 1], None, op0=OP.mult,
                )
                vp.append(t_)

            # ---------- qhT[e, t] = (Wq @ qT) / 8  (f32r) ----------
            qh = []
            for e in range(ND):
                t_ = main.tile([P, T], f32r, tag=f"qh{e}", name=f"qh{e}")
                ps = psmm.tile([P, T], f32, tag="mm")
                for dc in range(ND):
                    nc.tensor.matmul(
                        ps, lhsT=wq[:, dc, ts(e, P)], rhs=qTr[dc][:],
                        start=(dc == 0), stop=(dc == ND - 1),
                    )
                nc.vector.tensor_scalar(
                    t_[:], ps, float(1.0 / np.sqrt(DH)), None, op0=OP.mult
                )
                qh.append(t_)

            # ---------- transpose masks to [s, t] (bf16 PE transposes) -------
            # All 16 transposes of a 4-chunk group land in one psq tile, then
            # one wide DVE copy moves them to SBUF (avoids 1-deep PSUM
            # ping-pong between PE and the copier).
            mTq = []
            for g in range(2):
                mt = main.tile([P, 4, T], bf16, tag=f"sc{g}", name=f"mTq{g}")
                ps_t = psq.tile([P, 4, T], bf16, tag="q", name=f"pst{g}")
                for i in range(4):
                    for tt in range(NT):
                        nc.tensor.matmul(
                            ps_t[:, i, ts(tt, P)],
                            lhsT=mask01[tt][:, ts(4 * g + i, P)],
                            rhs=ident, is_transpose=True, skip_group_check=True,
                        )
                nc.vector.tensor_copy(mt[:], ps_t)
                mTq.append(mt)

            # ---------- attention: per 256-token half, quads of 4 chunks -----
            # ctxT[et][ro:ro+DH, t] per head; epilogue per half underneath the
            # other half's attention.
            cx_tags = ["sc2", "sc3", "wk0", "wk1"]
            ctxT = [
                main.tile([P, T], f32, tag=cx_tags[dt_i], name=f"cx{dt_i}")
                for dt_i in range(ND)
            ]
            oT_big = main.tile([P, ND, T], f32, tag="vtab", name="oT")
            oT = [oT_big[:, dt_i, :] for dt_i in range(ND)]

            def attention_half(half, interleave=None):
                tok = ds(half * HT, HT)
                for h in range(H):
                    et, ro = h // 2, (h % 2) * 64
                    if h % 2 == 0:
                        den_pair = scr2.tile([1, 2 * HT], f32r, tag="den")
                        ps_ctx2 = psctx.tile([DH + 1, 2, HT], f32, tag="ctx")
                    ps_ctx = ps_ctx2[:, h % 2, :]
                    for g in range(2):
                        ps_att = psq.tile([P, 4, HT], f32, tag="q")
                        for i in range(4):
                            nc.tensor.matmul(
                                ps_att[:, i, :],
                                lhsT=kp[et][ro : ro + DH, ts(4 * g + i, P)],
                                rhs=qh[et][ro : ro + DH, tok],
                                start=True, stop=True, skip_group_check=True,
                            )
                        w = scr2.tile([P, 4, HT], bf16, tag="u")
                        nc.scalar.activation(w[:], ps_att, AF.Exp)
                        nc.vector.tensor_tensor(
                            w[:], w[:], mTq[g][:, :, tok], OP.mult
                        )
                        for i in range(4):
                            nc.tensor.matmul(
                                ps_ctx, lhsT=vp[4 * g + i][:, h, :], rhs=w[:, i, :],
                                start=(g == 0 and i == 0), stop=(g == 1 and i == 3),
                                skip_group_check=True,
                            )
                    nc.vector.tensor_copy(
                        ctxT[et][ro : ro + DH, tok].bitcast(f32r), ps_ctx[0:DH, :]
                    )
                    nc.vector.reciprocal(
                        den_pair[0:1, ds((h % 2) * HT, HT)], ps_ctx[DH : DH + 1, :]
                    )
                    if h % 2 == 1:
                        ps_rb = psaux.tile([P, HT], f32, tag="aux")
                        nc.tensor.matmul(
                            ps_rb, lhsT=selA, rhs=den_pair[0:1, 0:HT],
                            start=True, stop=False,
                        )
                        nc.tensor.matmul(
                            ps_rb, lhsT=selB, rhs=den_pair[0:1, HT : 2 * HT],
                            start=False, stop=True,
                        )
                        nc.vector.tensor_tensor(
                            ctxT[et][:, tok].bitcast(f32r), ctxT[et][:, tok],
                            ps_rb, OP.mult,
                        )
                        if interleave is not None and interleave[h // 2] is not None:
                            interleave[h // 2]()

            def epilogue_parts(half):
                parts = []
                tok = ds(half * HT, HT)
                st = {}

                def part_a():
                    # oT[e, t] = Wo @ ctx.T (f32r)
                    for e in range(ND):
                        ps = psmm.tile([P, T], f32, tag="mm")
                        for dc in range(ND):
                            nc.tensor.matmul(
                                ps[:, 0:HT], lhsT=wo[:, dc, ts(e, P)],
                                rhs=ctxT[dc][:, tok].bitcast(f32r),
                                start=(dc == 0), stop=(dc == ND - 1),
                            )
                        nc.vector.tensor_copy(oT[e][:, tok].bitcast(f32r), ps[:, 0:HT])

                def part_b():
                    # LayerNorm stats via f32r ones-matmuls
                    ps_mu = psmm.tile([P, T], f32, tag="mm")
                    st["ps_mu"] = ps_mu
                    sqs = []
                    for dc in range(ND):
                        sq = scr2.tile([P, HT], f32r, tag="lnsq")
                        nc.gpsimd.tensor_tensor(
                            sq[:], oT[dc][:, tok], oT[dc][:, tok], OP.mult
                        )
                        sqs.append(sq)
                    for dc in range(ND):
                        nc.tensor.matmul(
                            ps_mu[0:1, 0:HT], lhsT=ones_col_r[:],
                            rhs=oT[dc][:, tok].bitcast(f32r),
                            start=(dc == 0), stop=(dc == ND - 1),
                            skip_group_check=True,
                        )
                    for dc in range(ND):
                        nc.tensor.matmul(
                            ps_mu[0:1, HT : 2 * HT], lhsT=ones_col_r[:],
                            rhs=sqs[dc][:],
                            start=(dc == 0), stop=(dc == ND - 1),
                            skip_group_check=True,
                        )

                def part_c():
                    ps_mu = st["ps_mu"]
                    mu_row = main.tile([1, HT], f32, tag="mu", name=f"mu{half}")
                    ms_row = main.tile([1, HT], f32, tag="ms", name=f"ms{half}")
                    nc.scalar.mul(mu_row[:], ps_mu[0:1, 0:HT], 1.0 / D)
                    nc.scalar.mul(ms_row[:], ps_mu[0:1, HT : 2 * HT], 1.0 / D)
                    var_row = main.tile([1, HT], f32, tag="var", name=f"var{half}")
                    nc.vector.tensor_tensor(var_row[:], mu_row[:], mu_row[:], OP.mult)
                    nc.vector.tensor_sub(var_row[:], ms_row[:], var_row[:])
                    sd_row = main.tile([1, HT], f32, tag="sd", name=f"sd{half}")
                    nc.scalar.activation(sd_row[:], var_row[:], AF.Sqrt, bias=eps_ln[:])
                    rstd_row = main.tile([1, HT], f32r, tag="rstd", name=f"rstd{half}")
                    nc.vector.reciprocal(rstd_row[:], sd_row[:])
                    crow = main.tile([1, HT], f32r, tag="crow", name=f"crow{half}")
                    nc.vector.scalar_tensor_tensor(
                        crow[:], mu_row[:], -1.0, rstd_row[:].bitcast(f32),
                        op0=OP.mult, op1=OP.mult
                    )
                    ps_b = psaux.tile([P, T], f32, tag="aux")
                    nc.tensor.matmul(
                        ps_b[:, 0:HT], lhsT=ones_row_r, rhs=rstd_row[:],
                        start=True, stop=True, skip_group_check=True,
                    )
                    nc.tensor.matmul(
                        ps_b[:, HT : 2 * HT], lhsT=ones_row_r, rhs=crow[:],
                        start=True, stop=True, skip_group_check=True,
                    )
                    nrm = []
                    for dt_i in range(ND):
                        nc.vector.tensor_tensor(
                            oT[dt_i][:, tok].bitcast(f32r), oT[dt_i][:, tok],
                            ps_b[:, 0:HT], OP.mult,
                        )
                        nc.vector.tensor_tensor(
                            oT[dt_i][:, tok].bitcast(f32r), oT[dt_i][:, tok],
                            ps_b[:, HT : 2 * HT], OP.add,
                        )
                        n_ = scr2.tile([P, HT], f32r, tag=f"nrm{dt_i % 2}")
                        nc.vector.scalar_tensor_tensor(
                            n_[:], oT[dt_i][:, tok], g_sb[:, dt_i : dt_i + 1],
                            b_sb[:, dt_i : dt_i + 1].to_broadcast([P, HT]),
                            op0=OP.mult, op1=OP.add,
                        )
                        nrm.append(n_)
                    st["nrm"] = nrm

                def part_d():
                    nrm = st["nrm"]
                    for qt, (off, sz) in enumerate(QD_TILES):
                        ps = psmm.tile([P, T], f32, tag="mm")
                        for e in range(ND):
                            nc.tensor.matmul(
                                ps[:sz, 0:HT], lhsT=wout[:, e, ds(off, sz)],
                                rhs=nrm[e][:],
                                start=(e == 0), stop=(e == ND - 1),
                            )
                        ot_sb = scr2.tile([P, HT], f32, tag="ot")
                        nc.scalar.add(
                            ot_sb[:sz, :], ps[:sz, 0:HT], bout_sb[:sz, qt : qt + 1]
                        )
                        nc.sync.dma_start(
                            out_dram.ap()[ds(off, sz), ds(half * HT, HT)],
                            ot_sb[:sz, :],
                        )

                return [part_a, part_b, part_c, part_d]

            attention_half(0)
            parts0 = epilogue_parts(0)
            attention_half(1, interleave=[parts0[0], parts0[1], None, None])
            parts1 = epilogue_parts(1)
            parts0[2]()
            parts1[0]()
            parts0[3]()
            parts1[1]()
            parts1[2]()
            parts1[3]()

    nc.compile()
    return nc


def _prep_in_maps(inputs):
    def c(a):
        return np.ascontiguousarray(a, dtype=np.float32)

    q = np.asarray(inputs["query_states"], dtype=np.float32).reshape(B * N, QD)
    shared = {
        "WqpT": c(np.asarray(inputs["Wqp"]).T),
        "WqT": c(np.asarray(inputs["Wq"]).T),
        "WkT": c(np.asarray(inputs["Wk"]).T),
        "WvT": c(np.asarray(inputs["Wv"]).T),
        "WoT": c(np.asarray(inputs["Wo"]).T),
        "WoutT": c(np.asarray(inputs["Wout"]).T),
        "memkT": c(np.asarray(inputs["mem_keys"]).T),
        "memkTr": c(np.asarray(inputs["mem_keys"]).T),
        "memvT": c(np.asarray(inputs["mem_values"]).T),
        "ln_g": c(np.asarray(inputs["ln_g"])),
        "ln_b": c(np.asarray(inputs["ln_b"])),
        "bout": c(np.pad(np.asarray(inputs["bout"]), (0, 384 - QD))),
    }
    in_maps = []
    for core in range(NCORES):
        m = dict(shared)
        m["queryT"] = c(q[core * T : (core + 1) * T, :].T)
        in_maps.append(m)
    return in_maps


def kernel(**inputs) -> np.ndarray:
    if "nc" not in _CACHE:
        _CACHE["nc"] = _build_nc()
    nc = _CACHE["nc"]
    in_maps = _prep_in_maps(inputs)
    res = run_bass_kernel_spmd(nc, in_maps, core_ids=list(range(NCORES)))
    out = np.empty((B * N, QD), dtype=np.float32)
    for core in range(NCORES):
        out[core * T : (core + 1) * T, :] = res.results[core]["outT"].T
    return out.reshape(B, N, QD)
